# revision 20
# baseline (speedup 1.0000x reference)
"""Trainium2 Bass kernel for nn_Attention_25915832664752.

Reference computation (per reference.py):
    For b in {Q,K,V}:  q0 = relu(IN(conv1d(Z, W[b,0])));  q1 = relu(IN(conv1d(Z, W[b,1]) @ L))
                       X_b = q0 + q1                                  [2048, 48]
    A  = exp(Q @ K^T)                                                 [2048, 2048]
    P  = A / rowsum(A);  Aa = (P + P^T)/2;  out = Aa @ V              [2048, 48]

Strategy (8 NeuronCores, tensor-parallel over nhid):
    Core c owns output channels [c*256, (c+1)*256).  W is pre-transposed on the
    host into a per-core streaming slab Wt[kt, p, o] with contraction index
    k = t*2048 + i on the partition axis, so the conv becomes a pure stream of
    128x128 stationary (lhsT=W^T tile) x [128, 48] moving (shifted Z window)
    matmuls accumulated in PSUM — W (113 MB/core) is read from HBM exactly once.
    The "@ L" branch is folded into the conv by precomputing ZcolL = Zcol @ L
    on-chip (Zcol never materialized; its tiles are slices of padded Z).
    After the convs: instance-norm + relu fused into one scalar-engine
    activation per tile; K and V are all-gathered; each core computes its
    row-block A_loc = exp(Q_loc K_full^T) and the transposed block
    At = exp(K_full Q_loc^T); then
        out = 0.5*rinv*(At^T-contract V_full) + 0.5*ReduceScatter(A_loc^T
              row-scaled V_loc)
    which realizes the symmetrized row-normalized attention exactly.
"""

import os
import sys

import numpy as np

sys.path.insert(0, "/opt/trn_rl_repo")

import orjson

import concourse.bass as bass
import concourse.mybir as mybir
from concourse import masks, tile
from concourse.bass_utils import run_bass_kernel_spmd

# ---------------------------------------------------------------- waitfix ---
# This neuronxcc build allows only ONE sync wait per instruction;
# TileContext emits instructions with several.  Rewrite the serialized BIR:
# hoist extra waits onto standalone NoOps inserted just before the
# instruction on the same engine (cumulative thresholds -> semantics kept).

_DMA_OPCODES = {
    "DMACopy", "DMATranspose", "TensorLoad", "TensorSave",
    "TriggeredCopy", "CollectiveCompute",
}
_wfix_counter = [0]


def _fix_block(instructions):
    out = []
    for ins in instructions:
        si = ins.get("sync_info")
        if not si:
            out.append(ins)
            continue
        waits = si.get("on_wait") or []
        updates = si.get("on_update") or []
        if len(waits) > 1:
            for w in waits[1:]:
                _wfix_counter[0] += 1
                out.append({
                    "engine": ins["engine"], "ins": [],
                    "name": f"WFIX-{_wfix_counter[0]}", "opcode": "NoOp",
                    "outs": [],
                    "sync_info": {"on_update": [], "on_wait": [w]},
                })
            si["on_wait"] = waits[:1]
        deferred = []
        if len(updates) > 1:
            assert ins.get("opcode", "") not in _DMA_OPCODES, (
                f"multi-update on DMA opcode: {ins['name']}"
            )
            si["on_update"] = updates[:1]
            for u in updates[1:]:
                _wfix_counter[0] += 1
                deferred.append({
                    "engine": ins["engine"], "ins": [],
                    "name": f"WFIX-{_wfix_counter[0]}", "opcode": "NoOp",
                    "outs": [],
                    "sync_info": {"on_update": [u], "on_wait": []},
                })
        out.append(ins)
        out.extend(deferred)
    return out


def _fix_bir_json_bytes(data: bytes) -> bytes:
    d = orjson.loads(data)
    for func in d.get("functions", []):
        for bb in func.get("blocks", []):
            bb["instructions"] = _fix_block(bb["instructions"])
    return orjson.dumps(d)


if not getattr(bass.Bass, "_waitfix_installed", False):
    _orig_to_json_bytes = bass.Bass.to_json_bytes

    def _patched_to_json_bytes(self) -> bytes:
        return _fix_bir_json_bytes(_orig_to_json_bytes(self))

    bass.Bass.to_json_bytes = _patched_to_json_bytes
    bass.Bass._waitfix_installed = True

# Synthesize the missing ``antenv.axon_hooks`` module so that
# ``run_bass_kernel_spmd(trace=True)`` can drive NTFF profiling through the
# axon PJRT plugin (the boot-time registration degrades silently when the
# module is absent).  Harmless when tracing is never requested.
try:
    import types

    import antenv

    if not hasattr(antenv, "axon_hooks"):
        _hooks_mod = types.ModuleType("antenv.axon_hooks")
        _ntff_hook = [None]
        _hooks_mod.set_axon_ntff_profile_hook = lambda h: _ntff_hook.__setitem__(0, h)
        _hooks_mod.get_axon_ntff_profile_hook = lambda: _ntff_hook[0]
        sys.modules["antenv.axon_hooks"] = _hooks_mod
        antenv.axon_hooks = _hooks_mod
        from trn_agent_boot.trn_boot import _ntff_profile_via_ctypes

        _hooks_mod.set_axon_ntff_profile_hook(
            _ntff_profile_via_ctypes("/opt/axon/libaxon_pjrt.so"))

    import concourse.bass_utils as _bu

    _bu.upload_artifacts = lambda tmpdir: tmpdir  # no fish share in container
except Exception:  # pragma: no cover - profiling is best-effort
    pass

# ------------------------------------------------------------- constants ---

NHID = 2048
NOPEN = 2048
N = 48          # spatial length
KD = 9          # conv kernel width
PAD = 4
NP = N + 2 * PAD            # 56 padded spatial
EPS = 1e-5
CORES = 8
OLOC = NHID // CORES        # 256 output channels per core
NGRP = 6                    # (b, r) conv groups
OCOLS = NGRP * OLOC         # 1536 W^T columns per core
KTOT = KD * NOPEN           # 18432 contraction length
NKT = KTOT // 128           # 144 k-tiles
ISUB = NOPEN // 128         # 16 i-subtiles
CHUNK_KT = 4                # k-tiles per W DMA chunk (sweep B)
NCHUNK = NKT // CHUNK_KT    # 36 chunks (sweep B)
CKA = 2                     # k-tiles per W DMA chunk (sweep A, 1 MB)
NCHA = NKT // CKA           # 72 chunks (sweep A)
ACOLS = 4 * OLOC            # sweep A (Q,K): 1024 W^T cols per k-row
BCOLS = 2 * OLOC            # sweep B (V):    512 W^T cols per k-row
F32 = mybir.dt.float32
F32R = mybir.dt.float32r


DEBUG = bool(int(os.environ.get("KERNEL_DEBUG", "0")))


def _build_nc():
    nc = bass.Bass()

    wta_d = nc.declare_dram_parameter(
        "wta", [NCHA, 128, CKA * ACOLS], F32R, isOutput=False)
    wtb_d = nc.declare_dram_parameter(
        "wtb", [NCHUNK, 128, CHUNK_KT * BCOLS], F32R, isOutput=False)
    z_d = nc.declare_dram_parameter("z", [NOPEN, N], F32, isOutput=False)
    l_d = nc.declare_dram_parameter("l", [N, N], F32, isOutput=False)
    out_d = nc.declare_dram_parameter("out", [OLOC, N], F32, isOutput=True)
    if DEBUG:
        dbg_conv = nc.declare_dram_parameter(
            "dbg_conv", [12, 128, N], F32, isOutput=True)
        dbg_qkv = nc.declare_dram_parameter(
            "dbg_qkv", [3, 2, 128, N], F32, isOutput=True)
        dbg_rs = nc.declare_dram_parameter(
            "dbg_rs", [2, 128, 1], F32, isOutput=True)
        dbg_zcl = nc.declare_dram_parameter(
            "dbg_zcl", [128, NKT * N], F32, isOutput=True)

    with tile.TileContext(nc) as tc:
        with (
            tc.tile_pool(name="pers", bufs=1) as pers,
            tc.tile_pool(name="wpool", bufs=6) as wpool,
            tc.tile_pool(name="wpoolb", bufs=5) as wpoolb,
            tc.tile_pool(name="stats", bufs=1) as stats,
            tc.tile_pool(name="pacc", bufs=2, space="PSUM") as pacc,
            tc.tile_pool(name="ptrans", bufs=2, space="PSUM") as ptrans,
            tc.tile_pool(name="dram", bufs=1, space="DRAM") as dram,
        ):
            # ---------------- prologue: Z, L, identity, ZpadT, ZcolL -------
            ident = pers.tile([128, 128], F32, tag="ident")
            masks.make_identity(nc, ident[:])

            # rendezvous: absorb cross-core launch/progress skew while the
            # W stream runs, so the mid-kernel all-gathers don't stall
            identb = pers.tile([128, 128], mybir.dt.bfloat16, tag="identb")
            warmb = pers.tile([128, 512], mybir.dt.bfloat16, tag="warmb")
            nc.vector.tensor_copy(identb[:], ident[:])
            nc.vector.memset(warmb[:], 1.0)

            rg = [list(range(CORES))]
            rdv_in = dram.tile([2, 4], F32, tag="rdv_in")
            rdv_out = dram.tile([16, 4], F32, tag="rdv_out", addr_space="Shared")
            nc.gpsimd.collective_compute(
                "AllGather", mybir.AluOpType.bypass,
                replica_groups=rg, ins=[rdv_in.opt()], outs=[rdv_out.opt()])

            # padded Z: 16 tiles [128, 56] side by side
            zpad = pers.tile([128, ISUB * NP], F32, tag="zpad")
            nc.vector.memset(zpad[:], 0.0)
            zpad_v = zpad[:].rearrange("p (a c) -> p a c", c=NP)
            nc.sync.dma_start(
                out=zpad_v[:, :, PAD:PAD + N],
                in_=z_d[:].rearrange("(a p) n -> p a n", p=128),
            )

            # L padded into 9 shifted copies: lpad[t:t+48, t*48:(t+1)*48] = L
            lpad = pers.tile([128, KD * N], F32, tag="lpad")
            nc.vector.memset(lpad[:], 0.0)
            for t in range(KD):
                nc.sync.dma_start(out=lpad[t:t + N, t * N:(t + 1) * N], in_=l_d[:])

            # ZpadT [56, 2048] via PE transposes of the 16 padded tiles
            zpadT = pers.tile([128, NOPEN], F32, tag="zpadT")
            for s in range(ISUB):
                ps = ptrans.tile([128, 128], F32, tag="ptrans")
                nc.tensor.transpose(
                    ps[0:NP, :], zpad[:, s * NP:(s + 1) * NP], ident[:])
                nc.scalar.copy(zpadT[0:NP, s * 128:(s + 1) * 128], ps[0:NP, :])

            # F32R copy of zpad for the conv stationaries
            zpadr = pers.tile([128, ISUB * NP], F32R, tag="zpadr")
            nc.vector.tensor_copy(zpadr[:], zpad[:])

            # ZcolL[k, n'] = sum_n Zpad[i, n+t] L[n, n'] — batched per i-subtile
            # (all 9 shifts in one N=432 matmul); stored isub-major:
            # slice for (t, s) lives at (s*KD + t) * N
            zcolL = pers.tile([128, ISUB * KD * N], F32R, tag="zcolL")
            for s in range(ISUB):
                ps = ptrans.tile([128, KD * N], F32, tag="pzcl", name=f"pzcl{s}", bufs=2)
                nc.tensor.matmul(
                    ps[:, 0:KD * N],
                    zpadT[0:NP, s * 128:(s + 1) * 128],
                    lpad[0:NP, :],
                    start=True, stop=True,
                )
                # alternate engines to halve the copy-chain latency
                eng_copy = (nc.scalar.copy if s % 2 == 0
                            else nc.vector.tensor_copy)
                eng_copy(zcolL[:, s * KD * N:(s + 1) * KD * N], ps[:, 0:KD * N])

            if DEBUG:
                nc.sync.dma_start(out=dbg_zcl[:], in_=zcolL[:])

            # ---------------- conv: stream W as the MOVING operand ---------
            # lhsT (stationary) = [128, 48] Z window / ZcolL tile (40 ns
            # LDWEIGHTS); rhs = W^T columns streaming at 1 col/cycle.  Two
            # k-sweeps: A covers Q+K groups (cols [g0,g2 | g1,g3]), B covers V
            # (cols [g4 | g5]) so the K all-gather + attention prework hide
            # behind sweep B.  One PSUM bank per group accumulator
            # (start=True clears has_written for the whole bank).
            relu_sc = pers.tile([128, 12 * N], F32, tag="relu_sc")
            yt_sb = pers.tile([128, 6 * OLOC], F32, tag="yt_sb")
            qkv = [pers.tile([128, 2 * N], F32, tag=f"qkv{b}", name=f"qkv{b}")
                   for b in range(3)]

            def sweep_epilogue(entries, label):
                """entries: list of (g, acc_ap[48, 256]).  Transpose each
                half to [128, 48], then batched instance-norm stats (one
                vector op per stage across all slots) + fused relu."""
                nslot = 2 * len(entries)
                xc = stats.tile([128, nslot * N], F32, tag=f"xc{label}",
                                name=f"xc{label}")
                slots = []
                for idx, (g, acc_ap) in enumerate(entries):
                    nc.scalar.copy(
                        yt_sb[0:N, g * OLOC:(g + 1) * OLOC], acc_ap)
                    for h in range(2):
                        ot = g * 2 + h
                        slot = idx * 2 + h
                        ps2 = ptrans.tile([128, 128], F32, tag="ptrans",
                                          name=f"tp{ot}")
                        nc.tensor.transpose(
                            ps2[:, 0:N],
                            yt_sb[0:N, g * OLOC + h * 128:
                                  g * OLOC + (h + 1) * 128],
                            ident[0:N, 0:N])
                        nc.scalar.copy(xc[:, slot * N:(slot + 1) * N],
                                       ps2[:, 0:N])
                        if DEBUG:
                            nc.scalar.dma_start(
                                out=dbg_conv[ot],
                                in_=xc[:, slot * N:(slot + 1) * N])
                        slots.append((ot, slot))
                sm = stats.tile([128, nslot], F32, tag=f"sm{label}",
                                name=f"sm{label}")
                sq = stats.tile([128, nslot], F32, tag=f"sq{label}",
                                name=f"sq{label}")
                scr = stats.tile([128, nslot * N], F32, tag=f"scr{label}",
                                 name=f"scr{label}")
                for ot, slot in slots:
                    nc.vector.reduce_sum(
                        sm[:, slot:slot + 1], xc[:, slot * N:(slot + 1) * N],
                        axis=mybir.AxisListType.X)
                nc.vector.tensor_tensor(scr[:], xc[:], xc[:],
                                        op=mybir.AluOpType.mult)
                for ot, slot in slots:
                    nc.vector.reduce_sum(
                        sq[:, slot:slot + 1], scr[:, slot * N:(slot + 1) * N],
                        axis=mybir.AxisListType.X)
                mean = stats.tile([128, nslot], F32, tag=f"mean{label}",
                                  name=f"mean{label}")
                var = stats.tile([128, nslot], F32, tag=f"var{label}",
                                 name=f"var{label}")
                std = stats.tile([128, nslot], F32, tag=f"std{label}",
                                 name=f"std{label}")
                rsv = stats.tile([128, nslot], F32, tag=f"rsv{label}",
                                 name=f"rsv{label}")
                nb = stats.tile([128, nslot], F32, tag=f"nb{label}",
                                name=f"nb{label}")
                nc.vector.tensor_scalar_mul(mean[:], sm[:], 1.0 / N)
                nc.vector.tensor_scalar_mul(sq[:], sq[:], 1.0 / N)
                nc.vector.tensor_tensor(var[:], mean[:], mean[:],
                                        op=mybir.AluOpType.mult)
                nc.vector.tensor_tensor(var[:], sq[:], var[:],
                                        op=mybir.AluOpType.subtract)
                nc.vector.tensor_scalar_add(var[:], var[:], EPS)
                nc.scalar.sqrt(std[:], var[:])
                nc.vector.reciprocal(rsv[:], std[:])
                nc.vector.tensor_tensor(nb[:], mean[:], rsv[:],
                                        op=mybir.AluOpType.mult)
                nc.vector.tensor_scalar_mul(nb[:], nb[:], -1.0)
                for ot, slot in slots:
                    nc.scalar.activation(
                        relu_sc[:, ot * N:(ot + 1) * N],
                        xc[:, slot * N:(slot + 1) * N],
                        mybir.ActivationFunctionType.Relu,
                        bias=nb[:, slot:slot + 1], scale=rsv[:, slot:slot + 1])

            def qkv_add(b):
                for h in range(2):
                    ot0 = (2 * b) * 2 + h        # r = 0
                    ot1 = (2 * b + 1) * 2 + h    # r = 1
                    nc.vector.tensor_tensor(
                        qkv[b][:, h * N:(h + 1) * N],
                        relu_sc[:, ot0 * N:(ot0 + 1) * N],
                        relu_sc[:, ot1 * N:(ot1 + 1) * N],
                        op=mybir.AluOpType.add)
                    if DEBUG:
                        nc.scalar.dma_start(
                            out=dbg_qkv[b, h],
                            in_=qkv[b][:, h * N:(h + 1) * N])

            # ---- sweep A: Q + K (cols [g0,g2 | g1,g3], one N=512 matmul
            # per stationary per k-tile; acc r0/r1 each fill one PSUM bank)
            accA = [pacc.tile([128, 2 * OLOC], F32, tag="accw", name=f"accA{i}")
                    for i in range(2)]
            for gch in range(NCHA):
                wt = wpool.tile([128, CKA * ACOLS], F32R, tag="wt",
                                name=f"wta{gch}")
                nc.sync.dma_start(out=wt[:], in_=wta_d[gch])
                if gch % 2 == 0:
                    pw = ptrans.tile([128, 512], F32, tag="pzcl",
                                     name=f"warmA{gch}", bufs=2)
                    nc.tensor.matmul(pw[:, 0:512], identb[:], warmb[:],
                                     start=True, stop=True)
                for j in range(CKA):
                    kt = gch * CKA + j
                    t, s = kt // ISUB, kt % ISUB
                    lhs0 = zpadr[:, s * NP + t: s * NP + t + N]
                    lhs1 = zcolL[:, (s * KD + t) * N:(s * KD + t + 1) * N]
                    base = j * ACOLS
                    nc.tensor.matmul(
                        accA[0][0:N, :], lhs0, wt[:, base: base + 512],
                        start=(kt == 0), stop=(kt == NKT - 1))
                    nc.tensor.matmul(
                        accA[1][0:N, :], lhs1, wt[:, base + 512: base + 1024],
                        start=(kt == 0), stop=(kt == NKT - 1))

            # epilogue for the whole A sweep (K + Q), K all-gather next
            sweep_epilogue([(0, accA[0][0:N, 0:OLOC]),
                            (2, accA[0][0:N, OLOC:2 * OLOC]),
                            (1, accA[1][0:N, 0:OLOC]),
                            (3, accA[1][0:N, OLOC:2 * OLOC])], "A")
            qkv_add(1)
            qloc, kloc, vloc = qkv

            kb = dram.tile([OLOC, N], F32, tag="kb")
            vb = dram.tile([OLOC, N], F32, tag="vb")
            kg = dram.tile([NHID, N], F32, tag="kg", addr_space="Shared")
            vg = dram.tile([NHID, N], F32, tag="vg", addr_space="Shared")
            nc.scalar.dma_start(
                out=kb[:].rearrange("(a p) n -> p a n", p=128),
                in_=kloc[:].rearrange("p (a n) -> p a n", n=N))
            nc.gpsimd.collective_compute(
                "AllGather", mybir.AluOpType.bypass,
                replica_groups=rg, ins=[kb.opt()], outs=[kg.opt()])

            qkv_add(0)

            # qT / kT transposes; S and At overlap sweep B on the PE
            qT = pers.tile([128, 2 * 128], F32R, tag="qT")
            for h in range(2):
                ps = ptrans.tile([128, 128], F32, tag="ptrans")
                nc.tensor.transpose(
                    ps[0:N, :], qloc[:, h * N:(h + 1) * N], ident[:])
                nc.scalar.copy(qT[0:N, h * 128:(h + 1) * 128], ps[0:N, :])
            # ---- sweep B: V ----
            accB = [pacc.tile([128, OLOC], F32, tag="acc", name=f"accB{i}")
                    for i in range(2)]  # order: g4, g5

            def sweep_b(c0, c1):
                for gch in range(c0, c1):
                    wt = wpoolb.tile([128, CHUNK_KT * BCOLS], F32R, tag="wtb",
                                     name=f"wtb{gch}")
                    nc.sync.dma_start(out=wt[:], in_=wtb_d[gch])
                    pw = ptrans.tile([128, 512], F32, tag="pzcl",
                                     name=f"warmB{gch}", bufs=2)
                    nc.tensor.matmul(pw[:, 0:512], identb[:], warmb[:],
                                     start=True, stop=True)
                    for j in range(CHUNK_KT):
                        kt = gch * CHUNK_KT + j
                        t, s = kt // ISUB, kt % ISUB
                        lhs0 = zpadr[:, s * NP + t: s * NP + t + N]
                        lhs1 = zcolL[:, (s * KD + t) * N:(s * KD + t + 1) * N]
                        base = j * BCOLS
                        nc.tensor.matmul(
                            accB[0][0:N, :], lhs0,
                            wt[:, base: base + OLOC],
                            start=(kt == 0), stop=(kt == NKT - 1))
                        nc.tensor.matmul(
                            accB[1][0:N, :], lhs1,
                            wt[:, base + OLOC: base + 2 * OLOC],
                            start=(kt == 0), stop=(kt == NKT - 1))

            sweep_b(0, 18)

            kfull = pers.tile([128, 16 * N], F32, tag="kfull")
            nc.scalar.dma_start(
                out=kfull[:].rearrange("p (a n) -> p a n", n=N),
                in_=kg[:].rearrange("(a p) n -> p a n", p=128))
            kT = pers.tile([128, NHID], F32R, tag="kT")
            for jt in range(16):
                ps = ptrans.tile([128, 128], F32, tag="ptrans")
                nc.tensor.transpose(
                    ps[0:N, :], kfull[:, jt * N:(jt + 1) * N], ident[:])
                nc.scalar.copy(kT[0:N, jt * 128:(jt + 1) * 128], ps[0:N, :])

            # A = exp(Q K^T) rows + rowsum
            a_sb = [pers.tile([128, NHID], F32, tag=f"a{m}", name=f"a{m}")
                    for m in range(2)]
            rinvh = []
            for m in range(2):
                rspart = stats.tile([128, 4], F32, tag=f"rsp{m}", name=f"rsp{m}")
                for jc in range(4):
                    ps = ptrans.tile([128, 512], F32, tag="ptrans")
                    nc.tensor.matmul(
                        ps[:, 0:512],
                        qT[0:N, m * 128:(m + 1) * 128],
                        kT[0:N, jc * 512:(jc + 1) * 512],
                        start=True, stop=True)
                    nc.scalar.activation(
                        a_sb[m][:, jc * 512:(jc + 1) * 512], ps[:, 0:512],
                        mybir.ActivationFunctionType.Exp,
                        accum_out=rspart[:, jc:jc + 1])
                rowsum = stats.tile([128, 1], F32, tag=f"rowsum{m}", name=f"rowsum{m}")
                nc.vector.reduce_sum(rowsum[:], rspart[:], axis=mybir.AxisListType.X)
                rinv = stats.tile([128, 1], F32, tag=f"rinv{m}", name=f"rinv{m}")
                nc.vector.reciprocal(rinv[:], rowsum[:])
                rh = stats.tile([128, 1], F32, tag=f"rinvh{m}", name=f"rinvh{m}")
                nc.vector.tensor_scalar_mul(rh[:], rinv[:], 0.5)
                rinvh.append((rinv, rh))
                if DEBUG:
                    nc.scalar.dma_start(out=dbg_rs[m], in_=rowsum[:])

            # At = exp(K Q^T)  [2048, 256]
            at_sb = pers.tile([128, 16 * 256], F32, tag="at")
            for jt in range(16):
                ps = ptrans.tile([128, 256], F32, tag="ptrans")
                nc.tensor.matmul(
                    ps[:, 0:256],
                    kT[0:N, jt * 128:(jt + 1) * 128],
                    qT[0:N, 0:256],
                    start=True, stop=True)
                nc.scalar.activation(
                    at_sb[:, jt * 256:(jt + 1) * 256], ps[:, 0:256],
                    mybir.ActivationFunctionType.Exp)

            sweep_b(18, NCHUNK)

            sweep_epilogue([(4, accB[0][0:N, :]),
                            (5, accB[1][0:N, :])], "B")
            qkv_add(2)

            nc.scalar.dma_start(
                out=vb[:].rearrange("(a p) n -> p a n", p=128),
                in_=vloc[:].rearrange("p (a n) -> p a n", n=N))
            nc.gpsimd.collective_compute(
                "AllGather", mybir.AluOpType.bypass,
                replica_groups=rg, ins=[vb.opt()], outs=[vg.opt()])
            vfull = pers.tile([128, 16 * N], F32, tag="vfull")
            nc.scalar.dma_start(
                out=vfull[:].rearrange("p (a n) -> p a n", n=N),
                in_=vg[:].rearrange("(a p) n -> p a n", p=128))

            # ---------------- U = A_loc^T (rinv*V_loc); ReduceScatter ------
            vr = pers.tile([128, 2 * N], F32, tag="vr")
            for m in range(2):
                nc.vector.tensor_scalar_mul(
                    vr[:, m * N:(m + 1) * N], vloc[:, m * N:(m + 1) * N],
                    rinvh[m][0][:])
            u_sb = pers.tile([128, 16 * N], F32, tag="u")
            for jt in range(16):
                ps = ptrans.tile([128, 128], F32, tag="ptrans")
                for m in range(2):
                    nc.tensor.matmul(
                        ps[:, 0:N],
                        a_sb[m][:, jt * 128:(jt + 1) * 128],
                        vr[:, m * N:(m + 1) * N],
                        start=(m == 0), stop=(m == 1))
                nc.vector.tensor_copy(u_sb[:, jt * N:(jt + 1) * N], ps[:, 0:N])

            ub = dram.tile([NHID, N], F32, tag="ub")
            rsb = dram.tile([OLOC, N], F32, tag="rsb")
            nc.scalar.dma_start(
                out=ub[:].rearrange("(a p) n -> p a n", p=128),
                in_=u_sb[:].rearrange("p (a n) -> p a n", n=N))
            nc.gpsimd.collective_compute(
                "ReduceScatter", mybir.AluOpType.add,
                replica_groups=rg, ins=[ub.opt()], outs=[rsb.opt()])
            rs_sb = pers.tile([128, 2 * N], F32, tag="rs_sb")
            nc.scalar.dma_start(
                out=rs_sb[:].rearrange("p (a n) -> p a n", n=N),
                in_=rsb[:].rearrange("(a p) n -> p a n", p=128))

            # ---------------- out1 = rinv * (At^T-contract V_full) ---------
            fin = pers.tile([128, 2 * N], F32, tag="fin")
            rs_half = pers.tile([128, 2 * N], F32, tag="rs_half")
            for m in range(2):
                ps = ptrans.tile([128, 128], F32, tag="ptrans")
                for jt in range(16):
                    nc.tensor.matmul(
                        ps[:, 0:N],
                        at_sb[:, jt * 256 + m * 128: jt * 256 + (m + 1) * 128],
                        vfull[:, jt * N:(jt + 1) * N],
                        start=(jt == 0), stop=(jt == 15))
                nc.vector.tensor_scalar_mul(
                    rs_half[:, m * N:(m + 1) * N], rs_sb[:, m * N:(m + 1) * N], 0.5)
                nc.vector.scalar_tensor_tensor(
                    out=fin[:, m * N:(m + 1) * N],
                    in0=ps[:, 0:N],
                    scalar=rinvh[m][1][:],
                    in1=rs_half[:, m * N:(m + 1) * N],
                    op0=mybir.AluOpType.mult,
                    op1=mybir.AluOpType.add)

            nc.scalar.dma_start(
                out=out_d[:].rearrange("(a p) n -> p a n", p=128),
                in_=fin[:].rearrange("p (a n) -> p a n", n=N))

    return nc


_NC_CACHE = None


def _get_nc():
    global _NC_CACHE
    if _NC_CACHE is None:
        _NC_CACHE = _build_nc()
    return _NC_CACHE


def _prep_w(W: np.ndarray) -> list[np.ndarray]:
    """Per-core streaming slab: [NCHUNK, 128, CHUNK_KT*OCOLS] f32.

    col = (b*2+r)*256 + o_loc ; row k = t*2048 + i ; chunk-major partitions.
    """
    def sweep(wt, groups, cols, nch, ck):
        blk = np.concatenate(
            [wt[:, g * OLOC:(g + 1) * OLOC] for g in groups], axis=1)
        blk = blk.reshape(nch, ck, 128, cols).transpose(0, 2, 1, 3)
        return np.ascontiguousarray(
            blk.reshape(nch, 128, ck * cols), dtype=np.float32)

    shards = []
    for c in range(CORES):
        wc = W[:, :, c * OLOC:(c + 1) * OLOC, :, :]        # [3,2,256,2048,9]
        wt = wc.transpose(4, 3, 0, 1, 2).reshape(KTOT, OCOLS)
        shards.append((sweep(wt, [0, 2, 1, 3], ACOLS, NCHA, CKA),
                       sweep(wt, [4, 5], BCOLS, NCHUNK, CHUNK_KT)))
    return shards


def kernel(Z: np.ndarray, L: np.ndarray, W: np.ndarray) -> np.ndarray:
    nc = _get_nc()
    wts = _prep_w(np.asarray(W, dtype=np.float32))
    z = np.ascontiguousarray(Z, dtype=np.float32)
    l = np.ascontiguousarray(L, dtype=np.float32)
    in_maps = [{"wta": wts[c][0], "wtb": wts[c][1], "z": z, "l": l}
               for c in range(CORES)]
    trace = bool(int(os.environ.get("KERNEL_TRACE", "0")))
    kw = {}
    if trace and int(os.environ.get("KERNEL_TRACE_ALL", "0")):
        kw["trace_cores"] = list(range(CORES))
    res = run_bass_kernel_spmd(nc, in_maps, list(range(CORES)), trace=trace, **kw)
    kernel.last_result = res
    out = np.concatenate([res.results[c]["out"] for c in range(CORES)], axis=0)
    return out


# revision 21
# speedup vs baseline: 1.0196x; 1.0196x over previous
"""Trainium2 Bass kernel for nn_Attention_25915832664752.

Reference computation (per reference.py):
    For b in {Q,K,V}:  q0 = relu(IN(conv1d(Z, W[b,0])));  q1 = relu(IN(conv1d(Z, W[b,1]) @ L))
                       X_b = q0 + q1                                  [2048, 48]
    A  = exp(Q @ K^T)                                                 [2048, 2048]
    P  = A / rowsum(A);  Aa = (P + P^T)/2;  out = Aa @ V              [2048, 48]

Strategy (8 NeuronCores, tensor-parallel over nhid):
    Core c owns output channels [c*256, (c+1)*256).  W is pre-transposed on the
    host into a per-core streaming slab Wt[kt, p, o] with contraction index
    k = t*2048 + i on the partition axis, so the conv becomes a pure stream of
    128x128 stationary (lhsT=W^T tile) x [128, 48] moving (shifted Z window)
    matmuls accumulated in PSUM — W (113 MB/core) is read from HBM exactly once.
    The "@ L" branch is folded into the conv by precomputing ZcolL = Zcol @ L
    on-chip (Zcol never materialized; its tiles are slices of padded Z).
    After the convs: instance-norm + relu fused into one scalar-engine
    activation per tile; K and V are all-gathered; each core computes its
    row-block A_loc = exp(Q_loc K_full^T) and the transposed block
    At = exp(K_full Q_loc^T); then
        out = 0.5*rinv*(At^T-contract V_full) + 0.5*ReduceScatter(A_loc^T
              row-scaled V_loc)
    which realizes the symmetrized row-normalized attention exactly.
"""

import os
import sys

import numpy as np

sys.path.insert(0, "/opt/trn_rl_repo")

import orjson

import concourse.bass as bass
import concourse.mybir as mybir
from concourse import masks, tile
from concourse.bass_utils import run_bass_kernel_spmd

# ---------------------------------------------------------------- waitfix ---
# This neuronxcc build allows only ONE sync wait per instruction;
# TileContext emits instructions with several.  Rewrite the serialized BIR:
# hoist extra waits onto standalone NoOps inserted just before the
# instruction on the same engine (cumulative thresholds -> semantics kept).

_DMA_OPCODES = {
    "DMACopy", "DMATranspose", "TensorLoad", "TensorSave",
    "TriggeredCopy", "CollectiveCompute",
}
_wfix_counter = [0]


def _fix_block(instructions):
    out = []
    for ins in instructions:
        si = ins.get("sync_info")
        if not si:
            out.append(ins)
            continue
        waits = si.get("on_wait") or []
        updates = si.get("on_update") or []
        if len(waits) > 1:
            for w in waits[1:]:
                _wfix_counter[0] += 1
                out.append({
                    "engine": ins["engine"], "ins": [],
                    "name": f"WFIX-{_wfix_counter[0]}", "opcode": "NoOp",
                    "outs": [],
                    "sync_info": {"on_update": [], "on_wait": [w]},
                })
            si["on_wait"] = waits[:1]
        deferred = []
        if len(updates) > 1:
            assert ins.get("opcode", "") not in _DMA_OPCODES, (
                f"multi-update on DMA opcode: {ins['name']}"
            )
            si["on_update"] = updates[:1]
            for u in updates[1:]:
                _wfix_counter[0] += 1
                deferred.append({
                    "engine": ins["engine"], "ins": [],
                    "name": f"WFIX-{_wfix_counter[0]}", "opcode": "NoOp",
                    "outs": [],
                    "sync_info": {"on_update": [u], "on_wait": []},
                })
        out.append(ins)
        out.extend(deferred)
    return out


def _fix_bir_json_bytes(data: bytes) -> bytes:
    d = orjson.loads(data)
    for func in d.get("functions", []):
        for bb in func.get("blocks", []):
            bb["instructions"] = _fix_block(bb["instructions"])
    return orjson.dumps(d)


if not getattr(bass.Bass, "_waitfix_installed", False):
    _orig_to_json_bytes = bass.Bass.to_json_bytes

    def _patched_to_json_bytes(self) -> bytes:
        return _fix_bir_json_bytes(_orig_to_json_bytes(self))

    bass.Bass.to_json_bytes = _patched_to_json_bytes
    bass.Bass._waitfix_installed = True

# Synthesize the missing ``antenv.axon_hooks`` module so that
# ``run_bass_kernel_spmd(trace=True)`` can drive NTFF profiling through the
# axon PJRT plugin (the boot-time registration degrades silently when the
# module is absent).  Harmless when tracing is never requested.
try:
    import types

    import antenv

    if not hasattr(antenv, "axon_hooks"):
        _hooks_mod = types.ModuleType("antenv.axon_hooks")
        _ntff_hook = [None]
        _hooks_mod.set_axon_ntff_profile_hook = lambda h: _ntff_hook.__setitem__(0, h)
        _hooks_mod.get_axon_ntff_profile_hook = lambda: _ntff_hook[0]
        sys.modules["antenv.axon_hooks"] = _hooks_mod
        antenv.axon_hooks = _hooks_mod
        from trn_agent_boot.trn_boot import _ntff_profile_via_ctypes

        _hooks_mod.set_axon_ntff_profile_hook(
            _ntff_profile_via_ctypes("/opt/axon/libaxon_pjrt.so"))

    import concourse.bass_utils as _bu

    _bu.upload_artifacts = lambda tmpdir: tmpdir  # no fish share in container
except Exception:  # pragma: no cover - profiling is best-effort
    pass

# ------------------------------------------------------------- constants ---

NHID = 2048
NOPEN = 2048
N = 48          # spatial length
KD = 9          # conv kernel width
PAD = 4
NP = N + 2 * PAD            # 56 padded spatial
EPS = 1e-5
CORES = 8
OLOC = NHID // CORES        # 256 output channels per core
NGRP = 6                    # (b, r) conv groups
OCOLS = NGRP * OLOC         # 1536 W^T columns per core
KTOT = KD * NOPEN           # 18432 contraction length
NKT = KTOT // 128           # 144 k-tiles
ISUB = NOPEN // 128         # 16 i-subtiles
CHUNK_KT = 4                # k-tiles per W DMA chunk (sweep B)
NCHUNK = NKT // CHUNK_KT    # 36 chunks (sweep B)
CKA = 2                     # k-tiles per W DMA chunk (sweep A, 1 MB)
NCHA = NKT // CKA           # 72 chunks (sweep A)
ACOLS = 4 * OLOC            # sweep A (Q,K): 1024 W^T cols per k-row
BCOLS = 2 * OLOC            # sweep B (V):    512 W^T cols per k-row
F32 = mybir.dt.float32
F32R = mybir.dt.float32r


DEBUG = bool(int(os.environ.get("KERNEL_DEBUG", "0")))


def _build_nc():
    nc = bass.Bass()

    wta_d = nc.declare_dram_parameter(
        "wta", [NCHA, 128, CKA * ACOLS], F32R, isOutput=False)
    wtb_d = nc.declare_dram_parameter(
        "wtb", [NCHUNK, 128, CHUNK_KT * BCOLS], F32R, isOutput=False)
    z_d = nc.declare_dram_parameter("z", [NOPEN, N], F32, isOutput=False)
    l_d = nc.declare_dram_parameter("l", [N, N], F32, isOutput=False)
    out_d = nc.declare_dram_parameter("out", [OLOC, N], F32, isOutput=True)
    if DEBUG:
        dbg_conv = nc.declare_dram_parameter(
            "dbg_conv", [12, 128, N], F32, isOutput=True)
        dbg_qkv = nc.declare_dram_parameter(
            "dbg_qkv", [3, 2, 128, N], F32, isOutput=True)
        dbg_rs = nc.declare_dram_parameter(
            "dbg_rs", [2, 128, 1], F32, isOutput=True)
        dbg_zcl = nc.declare_dram_parameter(
            "dbg_zcl", [128, NKT * N], F32, isOutput=True)

    with tile.TileContext(nc) as tc:
        with (
            tc.tile_pool(name="pers", bufs=1) as pers,
            tc.tile_pool(name="wpool", bufs=6) as wpool,
            tc.tile_pool(name="wpoolb", bufs=5) as wpoolb,
            tc.tile_pool(name="stats", bufs=1) as stats,
            tc.tile_pool(name="pacc", bufs=2, space="PSUM") as pacc,
            tc.tile_pool(name="ptrans", bufs=2, space="PSUM") as ptrans,
            tc.tile_pool(name="dram", bufs=1, space="DRAM") as dram,
        ):
            # ---------------- prologue: Z, L, identity, ZpadT, ZcolL -------
            ident = pers.tile([128, 128], F32, tag="ident")
            masks.make_identity(nc, ident[:])

            # rendezvous: absorb cross-core launch/progress skew while the
            # W stream runs, so the mid-kernel all-gathers don't stall
            rg = [list(range(CORES))]
            rdv_in = dram.tile([2, 4], F32, tag="rdv_in")
            rdv_out = dram.tile([16, 4], F32, tag="rdv_out", addr_space="Shared")
            nc.gpsimd.collective_compute(
                "AllGather", mybir.AluOpType.bypass,
                replica_groups=rg, ins=[rdv_in.opt()], outs=[rdv_out.opt()])

            # padded Z: 16 tiles [128, 56] side by side
            zpad = pers.tile([128, ISUB * NP], F32, tag="zpad")
            nc.vector.memset(zpad[:], 0.0)
            zpad_v = zpad[:].rearrange("p (a c) -> p a c", c=NP)
            nc.sync.dma_start(
                out=zpad_v[:, :, PAD:PAD + N],
                in_=z_d[:].rearrange("(a p) n -> p a n", p=128),
            )

            # L padded into 9 shifted copies: lpad[t:t+48, t*48:(t+1)*48] = L
            lpad = pers.tile([128, KD * N], F32, tag="lpad")
            nc.vector.memset(lpad[:], 0.0)
            for t in range(KD):
                nc.sync.dma_start(out=lpad[t:t + N, t * N:(t + 1) * N], in_=l_d[:])

            # ZpadT [56, 2048] via PE transposes of the 16 padded tiles
            zpadT = pers.tile([128, NOPEN], F32, tag="zpadT")
            for s in range(ISUB):
                ps = ptrans.tile([128, 128], F32, tag="ptrans")
                nc.tensor.transpose(
                    ps[0:NP, :], zpad[:, s * NP:(s + 1) * NP], ident[:])
                zp_copy = (nc.scalar.copy if s % 2 == 0
                           else nc.vector.tensor_copy)
                zp_copy(zpadT[0:NP, s * 128:(s + 1) * 128], ps[0:NP, :])

            # F32R copy of zpad for the conv stationaries
            zpadr = pers.tile([128, ISUB * NP], F32R, tag="zpadr")
            nc.vector.tensor_copy(zpadr[:], zpad[:])

            # ZcolL[k, n'] = sum_n Zpad[i, n+t] L[n, n'] — batched per i-subtile
            # (all 9 shifts in one N=432 matmul); stored isub-major:
            # slice for (t, s) lives at (s*KD + t) * N
            zcolL = pers.tile([128, ISUB * KD * N], F32R, tag="zcolL")
            for s in range(ISUB):
                ps = ptrans.tile([128, KD * N], F32, tag="pzcl", name=f"pzcl{s}", bufs=2)
                nc.tensor.matmul(
                    ps[:, 0:KD * N],
                    zpadT[0:NP, s * 128:(s + 1) * 128],
                    lpad[0:NP, :],
                    start=True, stop=True,
                )
                # alternate engines to halve the copy-chain latency
                eng_copy = (nc.scalar.copy if s % 2 == 0
                            else nc.vector.tensor_copy)
                eng_copy(zcolL[:, s * KD * N:(s + 1) * KD * N], ps[:, 0:KD * N])

            if DEBUG:
                nc.sync.dma_start(out=dbg_zcl[:], in_=zcolL[:])

            # ---------------- conv: stream W as the MOVING operand ---------
            # lhsT (stationary) = [128, 48] Z window / ZcolL tile (40 ns
            # LDWEIGHTS); rhs = W^T columns streaming at 1 col/cycle.  Two
            # k-sweeps: A covers Q+K groups (cols [g0,g2 | g1,g3]), B covers V
            # (cols [g4 | g5]) so the K all-gather + attention prework hide
            # behind sweep B.  One PSUM bank per group accumulator
            # (start=True clears has_written for the whole bank).
            relu_sc = pers.tile([128, 12 * N], F32, tag="relu_sc")
            yt_sb = pers.tile([128, 6 * OLOC], F32, tag="yt_sb")
            qkv = [pers.tile([128, 2 * N], F32, tag=f"qkv{b}", name=f"qkv{b}")
                   for b in range(3)]

            def sweep_epilogue(entries, label):
                """entries: list of (g, acc_ap[48, 256]).  Transpose each
                half to [128, 48], then batched instance-norm stats (one
                vector op per stage across all slots) + fused relu."""
                nslot = 2 * len(entries)
                xc = stats.tile([128, nslot * N], F32, tag=f"xc{label}",
                                name=f"xc{label}")
                slots = []
                for idx, (g, acc_ap) in enumerate(entries):
                    nc.scalar.copy(
                        yt_sb[0:N, g * OLOC:(g + 1) * OLOC], acc_ap)
                    for h in range(2):
                        ot = g * 2 + h
                        slot = idx * 2 + h
                        ps2 = ptrans.tile([128, 128], F32, tag="ptrans",
                                          name=f"tp{ot}")
                        nc.tensor.transpose(
                            ps2[:, 0:N],
                            yt_sb[0:N, g * OLOC + h * 128:
                                  g * OLOC + (h + 1) * 128],
                            ident[0:N, 0:N])
                        nc.scalar.copy(xc[:, slot * N:(slot + 1) * N],
                                       ps2[:, 0:N])
                        if DEBUG:
                            nc.scalar.dma_start(
                                out=dbg_conv[ot],
                                in_=xc[:, slot * N:(slot + 1) * N])
                        slots.append((ot, slot))
                sm = stats.tile([128, nslot], F32, tag=f"sm{label}",
                                name=f"sm{label}")
                sq = stats.tile([128, nslot], F32, tag=f"sq{label}",
                                name=f"sq{label}")
                scr = stats.tile([128, nslot * N], F32, tag=f"scr{label}",
                                 name=f"scr{label}")
                for ot, slot in slots:
                    nc.vector.reduce_sum(
                        sm[:, slot:slot + 1], xc[:, slot * N:(slot + 1) * N],
                        axis=mybir.AxisListType.X)
                nc.vector.tensor_tensor(scr[:], xc[:], xc[:],
                                        op=mybir.AluOpType.mult)
                for ot, slot in slots:
                    nc.vector.reduce_sum(
                        sq[:, slot:slot + 1], scr[:, slot * N:(slot + 1) * N],
                        axis=mybir.AxisListType.X)
                mean = stats.tile([128, nslot], F32, tag=f"mean{label}",
                                  name=f"mean{label}")
                var = stats.tile([128, nslot], F32, tag=f"var{label}",
                                 name=f"var{label}")
                std = stats.tile([128, nslot], F32, tag=f"std{label}",
                                 name=f"std{label}")
                rsv = stats.tile([128, nslot], F32, tag=f"rsv{label}",
                                 name=f"rsv{label}")
                nb = stats.tile([128, nslot], F32, tag=f"nb{label}",
                                name=f"nb{label}")
                nc.vector.tensor_scalar_mul(mean[:], sm[:], 1.0 / N)
                nc.vector.tensor_scalar_mul(sq[:], sq[:], 1.0 / N)
                nc.vector.tensor_tensor(var[:], mean[:], mean[:],
                                        op=mybir.AluOpType.mult)
                nc.vector.tensor_tensor(var[:], sq[:], var[:],
                                        op=mybir.AluOpType.subtract)
                nc.vector.tensor_scalar_add(var[:], var[:], EPS)
                nc.scalar.sqrt(std[:], var[:])
                nc.vector.reciprocal(rsv[:], std[:])
                nc.vector.tensor_tensor(nb[:], mean[:], rsv[:],
                                        op=mybir.AluOpType.mult)
                nc.vector.tensor_scalar_mul(nb[:], nb[:], -1.0)
                for ot, slot in slots:
                    nc.scalar.activation(
                        relu_sc[:, ot * N:(ot + 1) * N],
                        xc[:, slot * N:(slot + 1) * N],
                        mybir.ActivationFunctionType.Relu,
                        bias=nb[:, slot:slot + 1], scale=rsv[:, slot:slot + 1])

            def qkv_add(b):
                for h in range(2):
                    ot0 = (2 * b) * 2 + h        # r = 0
                    ot1 = (2 * b + 1) * 2 + h    # r = 1
                    nc.vector.tensor_tensor(
                        qkv[b][:, h * N:(h + 1) * N],
                        relu_sc[:, ot0 * N:(ot0 + 1) * N],
                        relu_sc[:, ot1 * N:(ot1 + 1) * N],
                        op=mybir.AluOpType.add)
                    if DEBUG:
                        nc.scalar.dma_start(
                            out=dbg_qkv[b, h],
                            in_=qkv[b][:, h * N:(h + 1) * N])

            # ---- sweep A: Q + K (cols [g0,g2 | g1,g3], one N=512 matmul
            # per stationary per k-tile; acc r0/r1 each fill one PSUM bank)
            accA = [pacc.tile([128, 2 * OLOC], F32, tag="accw", name=f"accA{i}")
                    for i in range(2)]
            for gch in range(NCHA):
                wt = wpool.tile([128, CKA * ACOLS], F32R, tag="wt",
                                name=f"wta{gch}")
                nc.sync.dma_start(out=wt[:], in_=wta_d[gch])
                for j in range(CKA):
                    kt = gch * CKA + j
                    t, s = kt // ISUB, kt % ISUB
                    lhs0 = zpadr[:, s * NP + t: s * NP + t + N]
                    lhs1 = zcolL[:, (s * KD + t) * N:(s * KD + t + 1) * N]
                    base = j * ACOLS
                    nc.tensor.matmul(
                        accA[0][0:N, :], lhs0, wt[:, base: base + 512],
                        start=(kt == 0), stop=(kt == NKT - 1))
                    nc.tensor.matmul(
                        accA[1][0:N, :], lhs1, wt[:, base + 512: base + 1024],
                        start=(kt == 0), stop=(kt == NKT - 1))

            # epilogue for the whole A sweep (K + Q), K all-gather next
            sweep_epilogue([(0, accA[0][0:N, 0:OLOC]),
                            (2, accA[0][0:N, OLOC:2 * OLOC]),
                            (1, accA[1][0:N, 0:OLOC]),
                            (3, accA[1][0:N, OLOC:2 * OLOC])], "A")
            qkv_add(1)
            qloc, kloc, vloc = qkv

            kb = dram.tile([OLOC, N], F32, tag="kb")
            vb = dram.tile([OLOC, N], F32, tag="vb")
            kg = dram.tile([NHID, N], F32, tag="kg", addr_space="Shared")
            vg = dram.tile([NHID, N], F32, tag="vg", addr_space="Shared")
            nc.scalar.dma_start(
                out=kb[:].rearrange("(a p) n -> p a n", p=128),
                in_=kloc[:].rearrange("p (a n) -> p a n", n=N))
            nc.gpsimd.collective_compute(
                "AllGather", mybir.AluOpType.bypass,
                replica_groups=rg, ins=[kb.opt()], outs=[kg.opt()])

            qkv_add(0)

            # ---- sweep B: V ----
            accB = [pacc.tile([128, OLOC], F32, tag="acc", name=f"accB{i}")
                    for i in range(2)]  # order: g4, g5

            def sweep_b(c0, c1):
                for gch in range(c0, c1):
                    wt = wpoolb.tile([128, CHUNK_KT * BCOLS], F32R, tag="wtb",
                                     name=f"wtb{gch}")
                    nc.sync.dma_start(out=wt[:], in_=wtb_d[gch])
                    for j in range(CHUNK_KT):
                        kt = gch * CHUNK_KT + j
                        t, s = kt // ISUB, kt % ISUB
                        lhs0 = zpadr[:, s * NP + t: s * NP + t + N]
                        lhs1 = zcolL[:, (s * KD + t) * N:(s * KD + t + 1) * N]
                        base = j * BCOLS
                        nc.tensor.matmul(
                            accB[0][0:N, :], lhs0,
                            wt[:, base: base + OLOC],
                            start=(kt == 0), stop=(kt == NKT - 1))
                        nc.tensor.matmul(
                            accB[1][0:N, :], lhs1,
                            wt[:, base + OLOC: base + 2 * OLOC],
                            start=(kt == 0), stop=(kt == NKT - 1))

            sweep_b(0, 18)

            # qT / kT transposes; S and At overlap sweep B on the PE
            qT = pers.tile([128, 2 * 128], F32R, tag="qT")
            for h in range(2):
                ps = ptrans.tile([128, 128], F32, tag="ptrans")
                nc.tensor.transpose(
                    ps[0:N, :], qloc[:, h * N:(h + 1) * N], ident[:])
                nc.scalar.copy(qT[0:N, h * 128:(h + 1) * 128], ps[0:N, :])
            kfull = pers.tile([128, 16 * N], F32, tag="kfull")
            nc.gpsimd.dma_start(
                out=kfull[:].rearrange("p (a n) -> p a n", n=N),
                in_=kg[:].rearrange("(a p) n -> p a n", p=128))
            kT = pers.tile([128, NHID], F32R, tag="kT")
            for jt in range(16):
                ps = ptrans.tile([128, 128], F32, tag="ptrans")
                nc.tensor.transpose(
                    ps[0:N, :], kfull[:, jt * N:(jt + 1) * N], ident[:])
                nc.scalar.copy(kT[0:N, jt * 128:(jt + 1) * 128], ps[0:N, :])

            # A = exp(Q K^T) rows + rowsum
            a_sb = [pers.tile([128, NHID], F32, tag=f"a{m}", name=f"a{m}")
                    for m in range(2)]
            rinvh = []
            for m in range(2):
                rspart = stats.tile([128, 4], F32, tag=f"rsp{m}", name=f"rsp{m}")
                for jc in range(4):
                    ps = ptrans.tile([128, 512], F32, tag="ptrans")
                    nc.tensor.matmul(
                        ps[:, 0:512],
                        qT[0:N, m * 128:(m + 1) * 128],
                        kT[0:N, jc * 512:(jc + 1) * 512],
                        start=True, stop=True)
                    nc.scalar.activation(
                        a_sb[m][:, jc * 512:(jc + 1) * 512], ps[:, 0:512],
                        mybir.ActivationFunctionType.Exp,
                        accum_out=rspart[:, jc:jc + 1])
                rowsum = stats.tile([128, 1], F32, tag=f"rowsum{m}", name=f"rowsum{m}")
                nc.vector.reduce_sum(rowsum[:], rspart[:], axis=mybir.AxisListType.X)
                rinv = stats.tile([128, 1], F32, tag=f"rinv{m}", name=f"rinv{m}")
                nc.vector.reciprocal(rinv[:], rowsum[:])
                rh = stats.tile([128, 1], F32, tag=f"rinvh{m}", name=f"rinvh{m}")
                nc.vector.tensor_scalar_mul(rh[:], rinv[:], 0.5)
                rinvh.append((rinv, rh))
                if DEBUG:
                    nc.scalar.dma_start(out=dbg_rs[m], in_=rowsum[:])

            # At = exp(K Q^T)  [2048, 256]
            at_sb = pers.tile([128, 16 * 256], F32, tag="at")
            for jt in range(16):
                ps = ptrans.tile([128, 256], F32, tag="ptrans")
                nc.tensor.matmul(
                    ps[:, 0:256],
                    kT[0:N, jt * 128:(jt + 1) * 128],
                    qT[0:N, 0:256],
                    start=True, stop=True)
                nc.scalar.activation(
                    at_sb[:, jt * 256:(jt + 1) * 256], ps[:, 0:256],
                    mybir.ActivationFunctionType.Exp)

            sweep_b(18, NCHUNK)

            sweep_epilogue([(4, accB[0][0:N, :]),
                            (5, accB[1][0:N, :])], "B")
            qkv_add(2)

            nc.scalar.dma_start(
                out=vb[:].rearrange("(a p) n -> p a n", p=128),
                in_=vloc[:].rearrange("p (a n) -> p a n", n=N))
            nc.gpsimd.collective_compute(
                "AllGather", mybir.AluOpType.bypass,
                replica_groups=rg, ins=[vb.opt()], outs=[vg.opt()])
            vfull = pers.tile([128, 16 * N], F32, tag="vfull")
            nc.gpsimd.dma_start(
                out=vfull[:].rearrange("p (a n) -> p a n", n=N),
                in_=vg[:].rearrange("(a p) n -> p a n", p=128))

            # ---------------- U = A_loc^T (rinv*V_loc); ReduceScatter ------
            vr = pers.tile([128, 2 * N], F32, tag="vr")
            for m in range(2):
                nc.vector.tensor_scalar_mul(
                    vr[:, m * N:(m + 1) * N], vloc[:, m * N:(m + 1) * N],
                    rinvh[m][0][:])
            u_sb = pers.tile([128, 16 * N], F32, tag="u")
            for jt in range(16):
                ps = ptrans.tile([128, 128], F32, tag="ptrans")
                for m in range(2):
                    nc.tensor.matmul(
                        ps[:, 0:N],
                        a_sb[m][:, jt * 128:(jt + 1) * 128],
                        vr[:, m * N:(m + 1) * N],
                        start=(m == 0), stop=(m == 1))
                nc.vector.tensor_copy(u_sb[:, jt * N:(jt + 1) * N], ps[:, 0:N])

            ub = dram.tile([NHID, N], F32, tag="ub")
            rsb = dram.tile([OLOC, N], F32, tag="rsb")
            nc.scalar.dma_start(
                out=ub[:].rearrange("(a p) n -> p a n", p=128),
                in_=u_sb[:].rearrange("p (a n) -> p a n", n=N))
            nc.gpsimd.collective_compute(
                "ReduceScatter", mybir.AluOpType.add,
                replica_groups=rg, ins=[ub.opt()], outs=[rsb.opt()])
            rs_sb = pers.tile([128, 2 * N], F32, tag="rs_sb")
            nc.gpsimd.dma_start(
                out=rs_sb[:].rearrange("p (a n) -> p a n", n=N),
                in_=rsb[:].rearrange("(a p) n -> p a n", p=128))

            # ---------------- out1 = rinv * (At^T-contract V_full) ---------
            fin = pers.tile([128, 2 * N], F32, tag="fin")
            rs_half = pers.tile([128, 2 * N], F32, tag="rs_half")
            for m in range(2):
                ps = ptrans.tile([128, 128], F32, tag="ptrans")
                for jt in range(16):
                    nc.tensor.matmul(
                        ps[:, 0:N],
                        at_sb[:, jt * 256 + m * 128: jt * 256 + (m + 1) * 128],
                        vfull[:, jt * N:(jt + 1) * N],
                        start=(jt == 0), stop=(jt == 15))
                nc.vector.tensor_scalar_mul(
                    rs_half[:, m * N:(m + 1) * N], rs_sb[:, m * N:(m + 1) * N], 0.5)
                nc.vector.scalar_tensor_tensor(
                    out=fin[:, m * N:(m + 1) * N],
                    in0=ps[:, 0:N],
                    scalar=rinvh[m][1][:],
                    in1=rs_half[:, m * N:(m + 1) * N],
                    op0=mybir.AluOpType.mult,
                    op1=mybir.AluOpType.add)

            nc.scalar.dma_start(
                out=out_d[:].rearrange("(a p) n -> p a n", p=128),
                in_=fin[:].rearrange("p (a n) -> p a n", n=N))

    return nc


_NC_CACHE = None


def _get_nc():
    global _NC_CACHE
    if _NC_CACHE is None:
        _NC_CACHE = _build_nc()
    return _NC_CACHE


def _prep_w(W: np.ndarray) -> list[np.ndarray]:
    """Per-core streaming slab: [NCHUNK, 128, CHUNK_KT*OCOLS] f32.

    col = (b*2+r)*256 + o_loc ; row k = t*2048 + i ; chunk-major partitions.
    """
    def sweep(wt, groups, cols, nch, ck):
        blk = np.concatenate(
            [wt[:, g * OLOC:(g + 1) * OLOC] for g in groups], axis=1)
        blk = blk.reshape(nch, ck, 128, cols).transpose(0, 2, 1, 3)
        return np.ascontiguousarray(
            blk.reshape(nch, 128, ck * cols), dtype=np.float32)

    shards = []
    for c in range(CORES):
        wc = W[:, :, c * OLOC:(c + 1) * OLOC, :, :]        # [3,2,256,2048,9]
        wt = wc.transpose(4, 3, 0, 1, 2).reshape(KTOT, OCOLS)
        shards.append((sweep(wt, [0, 2, 1, 3], ACOLS, NCHA, CKA),
                       sweep(wt, [4, 5], BCOLS, NCHUNK, CHUNK_KT)))
    return shards


def kernel(Z: np.ndarray, L: np.ndarray, W: np.ndarray) -> np.ndarray:
    nc = _get_nc()
    wts = _prep_w(np.asarray(W, dtype=np.float32))
    z = np.ascontiguousarray(Z, dtype=np.float32)
    l = np.ascontiguousarray(L, dtype=np.float32)
    in_maps = [{"wta": wts[c][0], "wtb": wts[c][1], "z": z, "l": l}
               for c in range(CORES)]
    trace = bool(int(os.environ.get("KERNEL_TRACE", "0")))
    kw = {}
    if trace and int(os.environ.get("KERNEL_TRACE_ALL", "0")):
        kw["trace_cores"] = list(range(CORES))
    res = run_bass_kernel_spmd(nc, in_maps, list(range(CORES)), trace=trace, **kw)
    kernel.last_result = res
    out = np.concatenate([res.results[c]["out"] for c in range(CORES)], axis=0)
    return out


# revision 22
# speedup vs baseline: 1.5076x; 1.4785x over previous
"""Trainium2 Bass kernel for nn_Attention_25915832664752.

Reference computation (per reference.py):
    For b in {Q,K,V}:  q0 = relu(IN(conv1d(Z, W[b,0])));  q1 = relu(IN(conv1d(Z, W[b,1]) @ L))
                       X_b = q0 + q1                                  [2048, 48]
    A  = exp(Q @ K^T)                                                 [2048, 2048]
    P  = A / rowsum(A);  Aa = (P + P^T)/2;  out = Aa @ V              [2048, 48]

Strategy (8 NeuronCores, tensor-parallel over nhid):
    Core c owns output channels [c*256, (c+1)*256).  W is pre-transposed on the
    host into a per-core streaming slab Wt[kt, p, o] with contraction index
    k = t*2048 + i on the partition axis, so the conv becomes a pure stream of
    128x128 stationary (lhsT=W^T tile) x [128, 48] moving (shifted Z window)
    matmuls accumulated in PSUM — W (113 MB/core) is read from HBM exactly once.
    The "@ L" branch is folded into the conv by precomputing ZcolL = Zcol @ L
    on-chip (Zcol never materialized; its tiles are slices of padded Z).
    After the convs: instance-norm + relu fused into one scalar-engine
    activation per tile; K and V are all-gathered; each core computes its
    row-block A_loc = exp(Q_loc K_full^T) and the transposed block
    At = exp(K_full Q_loc^T); then
        out = 0.5*rinv*(At^T-contract V_full) + 0.5*ReduceScatter(A_loc^T
              row-scaled V_loc)
    which realizes the symmetrized row-normalized attention exactly.
"""

import os
import sys

import numpy as np

sys.path.insert(0, "/opt/trn_rl_repo")

import orjson

import concourse.bass as bass
import concourse.mybir as mybir
from concourse import masks, tile
from concourse.bass_utils import run_bass_kernel_spmd

# ---------------------------------------------------------------- waitfix ---
# This neuronxcc build allows only ONE sync wait per instruction;
# TileContext emits instructions with several.  Rewrite the serialized BIR:
# hoist extra waits onto standalone NoOps inserted just before the
# instruction on the same engine (cumulative thresholds -> semantics kept).

_DMA_OPCODES = {
    "DMACopy", "DMATranspose", "TensorLoad", "TensorSave",
    "TriggeredCopy", "CollectiveCompute",
}
_wfix_counter = [0]


def _fix_block(instructions):
    out = []
    for ins in instructions:
        si = ins.get("sync_info")
        if not si:
            out.append(ins)
            continue
        waits = si.get("on_wait") or []
        updates = si.get("on_update") or []
        if len(waits) > 1:
            for w in waits[1:]:
                _wfix_counter[0] += 1
                out.append({
                    "engine": ins["engine"], "ins": [],
                    "name": f"WFIX-{_wfix_counter[0]}", "opcode": "NoOp",
                    "outs": [],
                    "sync_info": {"on_update": [], "on_wait": [w]},
                })
            si["on_wait"] = waits[:1]
        deferred = []
        if len(updates) > 1:
            assert ins.get("opcode", "") not in _DMA_OPCODES, (
                f"multi-update on DMA opcode: {ins['name']}"
            )
            si["on_update"] = updates[:1]
            for u in updates[1:]:
                _wfix_counter[0] += 1
                deferred.append({
                    "engine": ins["engine"], "ins": [],
                    "name": f"WFIX-{_wfix_counter[0]}", "opcode": "NoOp",
                    "outs": [],
                    "sync_info": {"on_update": [u], "on_wait": []},
                })
        out.append(ins)
        out.extend(deferred)
    return out


def _fix_bir_json_bytes(data: bytes) -> bytes:
    d = orjson.loads(data)
    for func in d.get("functions", []):
        for bb in func.get("blocks", []):
            bb["instructions"] = _fix_block(bb["instructions"])
    return orjson.dumps(d)


if not getattr(bass.Bass, "_waitfix_installed", False):
    _orig_to_json_bytes = bass.Bass.to_json_bytes

    def _patched_to_json_bytes(self) -> bytes:
        return _fix_bir_json_bytes(_orig_to_json_bytes(self))

    bass.Bass.to_json_bytes = _patched_to_json_bytes
    bass.Bass._waitfix_installed = True

# Synthesize the missing ``antenv.axon_hooks`` module so that
# ``run_bass_kernel_spmd(trace=True)`` can drive NTFF profiling through the
# axon PJRT plugin (the boot-time registration degrades silently when the
# module is absent).  Harmless when tracing is never requested.
try:
    import types

    import antenv

    if not hasattr(antenv, "axon_hooks"):
        _hooks_mod = types.ModuleType("antenv.axon_hooks")
        _ntff_hook = [None]
        _hooks_mod.set_axon_ntff_profile_hook = lambda h: _ntff_hook.__setitem__(0, h)
        _hooks_mod.get_axon_ntff_profile_hook = lambda: _ntff_hook[0]
        sys.modules["antenv.axon_hooks"] = _hooks_mod
        antenv.axon_hooks = _hooks_mod
        from trn_agent_boot.trn_boot import _ntff_profile_via_ctypes

        _hooks_mod.set_axon_ntff_profile_hook(
            _ntff_profile_via_ctypes("/opt/axon/libaxon_pjrt.so"))

    import concourse.bass_utils as _bu

    _bu.upload_artifacts = lambda tmpdir: tmpdir  # no fish share in container
except Exception:  # pragma: no cover - profiling is best-effort
    pass

# ------------------------------------------------------------- constants ---

NHID = 2048
NOPEN = 2048
N = 48          # spatial length
KD = 9          # conv kernel width
PAD = 4
NP = N + 2 * PAD            # 56 padded spatial
EPS = 1e-5
CORES = 8
OLOC = NHID // CORES        # 256 output channels per core
NGRP = 6                    # (b, r) conv groups
OCOLS = NGRP * OLOC         # 1536 W^T columns per core
KTOT = KD * NOPEN           # 18432 contraction length
NKT = KTOT // 128           # 144 k-tiles
ISUB = NOPEN // 128         # 16 i-subtiles
CKA = 4                     # k-tiles per W DMA chunk (sweep A, 1 MB bf16)
NCHA = NKT // CKA           # 36 chunks (sweep A)
CKB = 8                     # k-tiles per W DMA chunk (sweep B, 1 MB bf16)
NCHB = NKT // CKB           # 18 chunks (sweep B)
ACOLS = 4 * OLOC            # sweep A (Q,K): 1024 W^T cols per k-row
BCOLS = 2 * OLOC            # sweep B (V):    512 W^T cols per k-row
F32 = mybir.dt.float32
F32R = mybir.dt.float32r
BF16 = mybir.dt.bfloat16


DEBUG = bool(int(os.environ.get("KERNEL_DEBUG", "0")))


def _build_nc():
    nc = bass.Bass()

    wta_d = nc.declare_dram_parameter(
        "wta", [NCHA, 128, CKA * ACOLS], BF16, isOutput=False)
    wtb_d = nc.declare_dram_parameter(
        "wtb", [NCHB, 128, CKB * BCOLS], BF16, isOutput=False)
    z_d = nc.declare_dram_parameter("z", [NOPEN, N], F32, isOutput=False)
    l_d = nc.declare_dram_parameter("l", [N, N], F32, isOutput=False)
    out_d = nc.declare_dram_parameter("out", [OLOC, N], F32, isOutput=True)
    if DEBUG:
        dbg_conv = nc.declare_dram_parameter(
            "dbg_conv", [12, 128, N], F32, isOutput=True)
        dbg_qkv = nc.declare_dram_parameter(
            "dbg_qkv", [3, 2, 128, N], F32, isOutput=True)
        dbg_rs = nc.declare_dram_parameter(
            "dbg_rs", [2, 128, 1], F32, isOutput=True)
        dbg_zcl = nc.declare_dram_parameter(
            "dbg_zcl", [128, NKT * N], F32, isOutput=True)

    with tile.TileContext(nc) as tc:
        with (
            tc.tile_pool(name="pers", bufs=1) as pers,
            tc.tile_pool(name="wpool", bufs=6) as wpool,
            tc.tile_pool(name="wpoolb", bufs=5) as wpoolb,
            tc.tile_pool(name="stats", bufs=1) as stats,
            tc.tile_pool(name="pacc", bufs=2, space="PSUM") as pacc,
            tc.tile_pool(name="ptrans", bufs=2, space="PSUM") as ptrans,
            tc.tile_pool(name="dram", bufs=1, space="DRAM") as dram,
        ):
            # ---------------- prologue: Z, L, identity, ZpadT, ZcolL -------
            ident = pers.tile([128, 128], F32, tag="ident")
            masks.make_identity(nc, ident[:])

            # rendezvous: absorb cross-core launch/progress skew while the
            # W stream runs, so the mid-kernel all-gathers don't stall
            rg = [list(range(CORES))]
            rdv_in = dram.tile([2, 4], F32, tag="rdv_in")
            rdv_out = dram.tile([16, 4], F32, tag="rdv_out", addr_space="Shared")
            nc.gpsimd.collective_compute(
                "AllGather", mybir.AluOpType.bypass,
                replica_groups=rg, ins=[rdv_in.opt()], outs=[rdv_out.opt()])

            # padded Z: 16 tiles [128, 56] side by side
            zpad = pers.tile([128, ISUB * NP], F32, tag="zpad")
            nc.vector.memset(zpad[:], 0.0)
            zpad_v = zpad[:].rearrange("p (a c) -> p a c", c=NP)
            nc.sync.dma_start(
                out=zpad_v[:, :, PAD:PAD + N],
                in_=z_d[:].rearrange("(a p) n -> p a n", p=128),
            )

            # L padded into 9 shifted copies: lpad[t:t+48, t*48:(t+1)*48] = L
            lpad = pers.tile([128, KD * N], F32, tag="lpad")
            nc.vector.memset(lpad[:], 0.0)
            for t in range(KD):
                nc.sync.dma_start(out=lpad[t:t + N, t * N:(t + 1) * N], in_=l_d[:])

            # ZpadT [56, 2048] via PE transposes of the 16 padded tiles
            zpadT = pers.tile([128, NOPEN], F32, tag="zpadT")
            for s in range(ISUB):
                ps = ptrans.tile([128, 128], F32, tag="ptrans")
                nc.tensor.transpose(
                    ps[0:NP, :], zpad[:, s * NP:(s + 1) * NP], ident[:])
                zp_copy = (nc.scalar.copy if s % 2 == 0
                           else nc.vector.tensor_copy)
                zp_copy(zpadT[0:NP, s * 128:(s + 1) * 128], ps[0:NP, :])

            # F32R copy of zpad for the conv stationaries
            zpadr = pers.tile([128, ISUB * NP], BF16, tag="zpadr")
            nc.vector.tensor_copy(zpadr[:], zpad[:])

            # ZcolL[k, n'] = sum_n Zpad[i, n+t] L[n, n'] — batched per i-subtile
            # (all 9 shifts in one N=432 matmul); stored isub-major:
            # slice for (t, s) lives at (s*KD + t) * N
            zcolL = pers.tile([128, ISUB * KD * N], BF16, tag="zcolL")
            for s in range(ISUB):
                ps = ptrans.tile([128, KD * N], F32, tag="pzcl", name=f"pzcl{s}", bufs=2)
                nc.tensor.matmul(
                    ps[:, 0:KD * N],
                    zpadT[0:NP, s * 128:(s + 1) * 128],
                    lpad[0:NP, :],
                    start=True, stop=True,
                )
                # alternate engines to halve the copy-chain latency
                eng_copy = (nc.scalar.copy if s % 2 == 0
                            else nc.vector.tensor_copy)
                eng_copy(zcolL[:, s * KD * N:(s + 1) * KD * N], ps[:, 0:KD * N])

            if DEBUG:
                nc.sync.dma_start(out=dbg_zcl[:], in_=zcolL[:])

            # ---------------- conv: stream W as the MOVING operand ---------
            # lhsT (stationary) = [128, 48] Z window / ZcolL tile (40 ns
            # LDWEIGHTS); rhs = W^T columns streaming at 1 col/cycle.  Two
            # k-sweeps: A covers Q+K groups (cols [g0,g2 | g1,g3]), B covers V
            # (cols [g4 | g5]) so the K all-gather + attention prework hide
            # behind sweep B.  One PSUM bank per group accumulator
            # (start=True clears has_written for the whole bank).
            relu_sc = pers.tile([128, 12 * N], F32, tag="relu_sc")
            yt_sb = pers.tile([128, 6 * OLOC], F32, tag="yt_sb")
            qkv = [pers.tile([128, 2 * N], F32, tag=f"qkv{b}", name=f"qkv{b}")
                   for b in range(3)]

            def sweep_epilogue(entries, label):
                """entries: list of (g, acc_ap[48, 256]).  Transpose each
                half to [128, 48], then batched instance-norm stats (one
                vector op per stage across all slots) + fused relu."""
                nslot = 2 * len(entries)
                xc = stats.tile([128, nslot * N], F32, tag=f"xc{label}",
                                name=f"xc{label}")
                slots = []
                for idx, (g, acc_ap) in enumerate(entries):
                    nc.scalar.copy(
                        yt_sb[0:N, g * OLOC:(g + 1) * OLOC], acc_ap)
                    for h in range(2):
                        ot = g * 2 + h
                        slot = idx * 2 + h
                        ps2 = ptrans.tile([128, 128], F32, tag="ptrans",
                                          name=f"tp{ot}")
                        nc.tensor.transpose(
                            ps2[:, 0:N],
                            yt_sb[0:N, g * OLOC + h * 128:
                                  g * OLOC + (h + 1) * 128],
                            ident[0:N, 0:N])
                        nc.scalar.copy(xc[:, slot * N:(slot + 1) * N],
                                       ps2[:, 0:N])
                        if DEBUG:
                            nc.scalar.dma_start(
                                out=dbg_conv[ot],
                                in_=xc[:, slot * N:(slot + 1) * N])
                        slots.append((ot, slot))
                sm = stats.tile([128, nslot], F32, tag=f"sm{label}",
                                name=f"sm{label}")
                sq = stats.tile([128, nslot], F32, tag=f"sq{label}",
                                name=f"sq{label}")
                scr = stats.tile([128, nslot * N], F32, tag=f"scr{label}",
                                 name=f"scr{label}")
                for ot, slot in slots:
                    nc.vector.reduce_sum(
                        sm[:, slot:slot + 1], xc[:, slot * N:(slot + 1) * N],
                        axis=mybir.AxisListType.X)
                nc.vector.tensor_tensor(scr[:], xc[:], xc[:],
                                        op=mybir.AluOpType.mult)
                for ot, slot in slots:
                    nc.vector.reduce_sum(
                        sq[:, slot:slot + 1], scr[:, slot * N:(slot + 1) * N],
                        axis=mybir.AxisListType.X)
                mean = stats.tile([128, nslot], F32, tag=f"mean{label}",
                                  name=f"mean{label}")
                var = stats.tile([128, nslot], F32, tag=f"var{label}",
                                 name=f"var{label}")
                std = stats.tile([128, nslot], F32, tag=f"std{label}",
                                 name=f"std{label}")
                rsv = stats.tile([128, nslot], F32, tag=f"rsv{label}",
                                 name=f"rsv{label}")
                nb = stats.tile([128, nslot], F32, tag=f"nb{label}",
                                name=f"nb{label}")
                nc.vector.tensor_scalar_mul(mean[:], sm[:], 1.0 / N)
                nc.vector.tensor_scalar_mul(sq[:], sq[:], 1.0 / N)
                nc.vector.tensor_tensor(var[:], mean[:], mean[:],
                                        op=mybir.AluOpType.mult)
                nc.vector.tensor_tensor(var[:], sq[:], var[:],
                                        op=mybir.AluOpType.subtract)
                nc.vector.tensor_scalar_add(var[:], var[:], EPS)
                nc.scalar.sqrt(std[:], var[:])
                nc.vector.reciprocal(rsv[:], std[:])
                nc.vector.tensor_tensor(nb[:], mean[:], rsv[:],
                                        op=mybir.AluOpType.mult)
                nc.vector.tensor_scalar_mul(nb[:], nb[:], -1.0)
                for ot, slot in slots:
                    nc.scalar.activation(
                        relu_sc[:, ot * N:(ot + 1) * N],
                        xc[:, slot * N:(slot + 1) * N],
                        mybir.ActivationFunctionType.Relu,
                        bias=nb[:, slot:slot + 1], scale=rsv[:, slot:slot + 1])

            def qkv_add(b):
                for h in range(2):
                    ot0 = (2 * b) * 2 + h        # r = 0
                    ot1 = (2 * b + 1) * 2 + h    # r = 1
                    nc.vector.tensor_tensor(
                        qkv[b][:, h * N:(h + 1) * N],
                        relu_sc[:, ot0 * N:(ot0 + 1) * N],
                        relu_sc[:, ot1 * N:(ot1 + 1) * N],
                        op=mybir.AluOpType.add)
                    if DEBUG:
                        nc.scalar.dma_start(
                            out=dbg_qkv[b, h],
                            in_=qkv[b][:, h * N:(h + 1) * N])

            # ---- sweep A: Q + K (cols [g0,g2 | g1,g3], one N=512 matmul
            # per stationary per k-tile; acc r0/r1 each fill one PSUM bank)
            accA = [pacc.tile([128, 2 * OLOC], F32, tag="accw", name=f"accA{i}")
                    for i in range(2)]
            for gch in range(NCHA):
                wt = wpool.tile([128, CKA * ACOLS], BF16, tag="wt",
                                name=f"wta{gch}")
                nc.sync.dma_start(out=wt[:], in_=wta_d[gch])
                for j in range(CKA):
                    kt = gch * CKA + j
                    t, s = kt // ISUB, kt % ISUB
                    lhs0 = zpadr[:, s * NP + t: s * NP + t + N]
                    lhs1 = zcolL[:, (s * KD + t) * N:(s * KD + t + 1) * N]
                    base = j * ACOLS
                    nc.tensor.matmul(
                        accA[0][0:N, :], lhs0, wt[:, base: base + 512],
                        start=(kt == 0), stop=(kt == NKT - 1))
                    nc.tensor.matmul(
                        accA[1][0:N, :], lhs1, wt[:, base + 512: base + 1024],
                        start=(kt == 0), stop=(kt == NKT - 1))

            # epilogue for the whole A sweep (K + Q), K all-gather next
            sweep_epilogue([(0, accA[0][0:N, 0:OLOC]),
                            (2, accA[0][0:N, OLOC:2 * OLOC]),
                            (1, accA[1][0:N, 0:OLOC]),
                            (3, accA[1][0:N, OLOC:2 * OLOC])], "A")
            qkv_add(1)
            qloc, kloc, vloc = qkv

            kb = dram.tile([OLOC, N], F32, tag="kb")
            vb = dram.tile([OLOC, N], F32, tag="vb")
            kg = dram.tile([NHID, N], F32, tag="kg", addr_space="Shared")
            vg = dram.tile([NHID, N], F32, tag="vg", addr_space="Shared")
            nc.scalar.dma_start(
                out=kb[:].rearrange("(a p) n -> p a n", p=128),
                in_=kloc[:].rearrange("p (a n) -> p a n", n=N))
            nc.gpsimd.collective_compute(
                "AllGather", mybir.AluOpType.bypass,
                replica_groups=rg, ins=[kb.opt()], outs=[kg.opt()])

            qkv_add(0)

            # ---- sweep B: V ----
            accB = [pacc.tile([128, OLOC], F32, tag="acc", name=f"accB{i}")
                    for i in range(2)]  # order: g4, g5

            def sweep_b(c0, c1):
                for gch in range(c0, c1):
                    wt = wpoolb.tile([128, CKB * BCOLS], BF16, tag="wtb",
                                     name=f"wtb{gch}")
                    nc.sync.dma_start(out=wt[:], in_=wtb_d[gch])
                    for j in range(CKB):
                        kt = gch * CKB + j
                        t, s = kt // ISUB, kt % ISUB
                        lhs0 = zpadr[:, s * NP + t: s * NP + t + N]
                        lhs1 = zcolL[:, (s * KD + t) * N:(s * KD + t + 1) * N]
                        base = j * BCOLS
                        nc.tensor.matmul(
                            accB[0][0:N, :], lhs0,
                            wt[:, base: base + OLOC],
                            start=(kt == 0), stop=(kt == NKT - 1))
                        nc.tensor.matmul(
                            accB[1][0:N, :], lhs1,
                            wt[:, base + OLOC: base + 2 * OLOC],
                            start=(kt == 0), stop=(kt == NKT - 1))

            sweep_b(0, 9)

            # qT / kT transposes; S and At overlap sweep B on the PE
            qT = pers.tile([128, 2 * 128], F32R, tag="qT")
            for h in range(2):
                ps = ptrans.tile([128, 128], F32, tag="ptrans")
                nc.tensor.transpose(
                    ps[0:N, :], qloc[:, h * N:(h + 1) * N], ident[:])
                nc.scalar.copy(qT[0:N, h * 128:(h + 1) * 128], ps[0:N, :])
            kfull = pers.tile([128, 16 * N], F32, tag="kfull")
            nc.gpsimd.dma_start(
                out=kfull[:].rearrange("p (a n) -> p a n", n=N),
                in_=kg[:].rearrange("(a p) n -> p a n", p=128))
            kT = pers.tile([128, NHID], F32R, tag="kT")
            for jt in range(16):
                ps = ptrans.tile([128, 128], F32, tag="ptrans")
                nc.tensor.transpose(
                    ps[0:N, :], kfull[:, jt * N:(jt + 1) * N], ident[:])
                nc.scalar.copy(kT[0:N, jt * 128:(jt + 1) * 128], ps[0:N, :])

            # A = exp(Q K^T) rows + rowsum
            a_sb = [pers.tile([128, NHID], F32, tag=f"a{m}", name=f"a{m}")
                    for m in range(2)]
            rinvh = []
            for m in range(2):
                rspart = stats.tile([128, 4], F32, tag=f"rsp{m}", name=f"rsp{m}")
                for jc in range(4):
                    ps = ptrans.tile([128, 512], F32, tag="ptrans")
                    nc.tensor.matmul(
                        ps[:, 0:512],
                        qT[0:N, m * 128:(m + 1) * 128],
                        kT[0:N, jc * 512:(jc + 1) * 512],
                        start=True, stop=True)
                    nc.scalar.activation(
                        a_sb[m][:, jc * 512:(jc + 1) * 512], ps[:, 0:512],
                        mybir.ActivationFunctionType.Exp,
                        accum_out=rspart[:, jc:jc + 1])
                rowsum = stats.tile([128, 1], F32, tag=f"rowsum{m}", name=f"rowsum{m}")
                nc.vector.reduce_sum(rowsum[:], rspart[:], axis=mybir.AxisListType.X)
                rinv = stats.tile([128, 1], F32, tag=f"rinv{m}", name=f"rinv{m}")
                nc.vector.reciprocal(rinv[:], rowsum[:])
                rh = stats.tile([128, 1], F32, tag=f"rinvh{m}", name=f"rinvh{m}")
                nc.vector.tensor_scalar_mul(rh[:], rinv[:], 0.5)
                rinvh.append((rinv, rh))
                if DEBUG:
                    nc.scalar.dma_start(out=dbg_rs[m], in_=rowsum[:])

            # At = exp(K Q^T)  [2048, 256]
            at_sb = pers.tile([128, 16 * 256], F32, tag="at")
            for jt in range(16):
                ps = ptrans.tile([128, 256], F32, tag="ptrans")
                nc.tensor.matmul(
                    ps[:, 0:256],
                    kT[0:N, jt * 128:(jt + 1) * 128],
                    qT[0:N, 0:256],
                    start=True, stop=True)
                nc.scalar.activation(
                    at_sb[:, jt * 256:(jt + 1) * 256], ps[:, 0:256],
                    mybir.ActivationFunctionType.Exp)

            sweep_b(9, NCHB)

            sweep_epilogue([(4, accB[0][0:N, :]),
                            (5, accB[1][0:N, :])], "B")
            qkv_add(2)

            nc.scalar.dma_start(
                out=vb[:].rearrange("(a p) n -> p a n", p=128),
                in_=vloc[:].rearrange("p (a n) -> p a n", n=N))
            nc.gpsimd.collective_compute(
                "AllGather", mybir.AluOpType.bypass,
                replica_groups=rg, ins=[vb.opt()], outs=[vg.opt()])
            vfull = pers.tile([128, 16 * N], F32, tag="vfull")
            nc.gpsimd.dma_start(
                out=vfull[:].rearrange("p (a n) -> p a n", n=N),
                in_=vg[:].rearrange("(a p) n -> p a n", p=128))

            # ---------------- U = A_loc^T (rinv*V_loc); ReduceScatter ------
            vr = pers.tile([128, 2 * N], F32, tag="vr")
            for m in range(2):
                nc.vector.tensor_scalar_mul(
                    vr[:, m * N:(m + 1) * N], vloc[:, m * N:(m + 1) * N],
                    rinvh[m][0][:])
            u_sb = pers.tile([128, 16 * N], F32, tag="u")
            for jt in range(16):
                ps = ptrans.tile([128, 128], F32, tag="ptrans")
                for m in range(2):
                    nc.tensor.matmul(
                        ps[:, 0:N],
                        a_sb[m][:, jt * 128:(jt + 1) * 128],
                        vr[:, m * N:(m + 1) * N],
                        start=(m == 0), stop=(m == 1))
                nc.vector.tensor_copy(u_sb[:, jt * N:(jt + 1) * N], ps[:, 0:N])

            ub = dram.tile([NHID, N], F32, tag="ub")
            rsb = dram.tile([OLOC, N], F32, tag="rsb")
            nc.scalar.dma_start(
                out=ub[:].rearrange("(a p) n -> p a n", p=128),
                in_=u_sb[:].rearrange("p (a n) -> p a n", n=N))
            nc.gpsimd.collective_compute(
                "ReduceScatter", mybir.AluOpType.add,
                replica_groups=rg, ins=[ub.opt()], outs=[rsb.opt()])
            rs_sb = pers.tile([128, 2 * N], F32, tag="rs_sb")
            nc.gpsimd.dma_start(
                out=rs_sb[:].rearrange("p (a n) -> p a n", n=N),
                in_=rsb[:].rearrange("(a p) n -> p a n", p=128))

            # ---------------- out1 = rinv * (At^T-contract V_full) ---------
            fin = pers.tile([128, 2 * N], F32, tag="fin")
            rs_half = pers.tile([128, 2 * N], F32, tag="rs_half")
            for m in range(2):
                ps = ptrans.tile([128, 128], F32, tag="ptrans")
                for jt in range(16):
                    nc.tensor.matmul(
                        ps[:, 0:N],
                        at_sb[:, jt * 256 + m * 128: jt * 256 + (m + 1) * 128],
                        vfull[:, jt * N:(jt + 1) * N],
                        start=(jt == 0), stop=(jt == 15))
                nc.vector.tensor_scalar_mul(
                    rs_half[:, m * N:(m + 1) * N], rs_sb[:, m * N:(m + 1) * N], 0.5)
                nc.vector.scalar_tensor_tensor(
                    out=fin[:, m * N:(m + 1) * N],
                    in0=ps[:, 0:N],
                    scalar=rinvh[m][1][:],
                    in1=rs_half[:, m * N:(m + 1) * N],
                    op0=mybir.AluOpType.mult,
                    op1=mybir.AluOpType.add)

            nc.scalar.dma_start(
                out=out_d[:].rearrange("(a p) n -> p a n", p=128),
                in_=fin[:].rearrange("p (a n) -> p a n", n=N))

    return nc


_NC_CACHE = None


def _get_nc():
    global _NC_CACHE
    if _NC_CACHE is None:
        _NC_CACHE = _build_nc()
    return _NC_CACHE


def _prep_w(W: np.ndarray) -> list[np.ndarray]:
    """Per-core streaming slab: [NCHUNK, 128, CHUNK_KT*OCOLS] f32.

    col = (b*2+r)*256 + o_loc ; row k = t*2048 + i ; chunk-major partitions.
    """
    import ml_dtypes

    def sweep(wt, groups, cols, nch, ck):
        blk = np.concatenate(
            [wt[:, g * OLOC:(g + 1) * OLOC] for g in groups], axis=1)
        blk = blk.reshape(nch, ck, 128, cols).transpose(0, 2, 1, 3)
        return np.ascontiguousarray(
            blk.reshape(nch, 128, ck * cols).astype(ml_dtypes.bfloat16))

    shards = []
    for c in range(CORES):
        wc = W[:, :, c * OLOC:(c + 1) * OLOC, :, :]        # [3,2,256,2048,9]
        wt = wc.transpose(4, 3, 0, 1, 2).reshape(KTOT, OCOLS)
        shards.append((sweep(wt, [0, 2, 1, 3], ACOLS, NCHA, CKA),
                       sweep(wt, [4, 5], BCOLS, NCHB, CKB)))
    return shards


def kernel(Z: np.ndarray, L: np.ndarray, W: np.ndarray) -> np.ndarray:
    nc = _get_nc()
    wts = _prep_w(np.asarray(W, dtype=np.float32))
    z = np.ascontiguousarray(Z, dtype=np.float32)
    l = np.ascontiguousarray(L, dtype=np.float32)
    in_maps = [{"wta": wts[c][0], "wtb": wts[c][1], "z": z, "l": l}
               for c in range(CORES)]
    trace = bool(int(os.environ.get("KERNEL_TRACE", "0")))
    kw = {}
    if trace and int(os.environ.get("KERNEL_TRACE_ALL", "0")):
        kw["trace_cores"] = list(range(CORES))
    res = run_bass_kernel_spmd(nc, in_maps, list(range(CORES)), trace=trace, **kw)
    kernel.last_result = res
    out = np.concatenate([res.results[c]["out"] for c in range(CORES)], axis=0)
    return out


# revision 23
# speedup vs baseline: 1.5171x; 1.0063x over previous
"""Trainium2 Bass kernel for nn_Attention_25915832664752.

Reference computation (per reference.py):
    For b in {Q,K,V}:  q0 = relu(IN(conv1d(Z, W[b,0])));  q1 = relu(IN(conv1d(Z, W[b,1]) @ L))
                       X_b = q0 + q1                                  [2048, 48]
    A  = exp(Q @ K^T)                                                 [2048, 2048]
    P  = A / rowsum(A);  Aa = (P + P^T)/2;  out = Aa @ V              [2048, 48]

Strategy (8 NeuronCores, tensor-parallel over nhid):
    Core c owns output channels [c*256, (c+1)*256).  W is pre-transposed on the
    host into a per-core streaming slab Wt[kt, p, o] with contraction index
    k = t*2048 + i on the partition axis, so the conv becomes a pure stream of
    128x128 stationary (lhsT=W^T tile) x [128, 48] moving (shifted Z window)
    matmuls accumulated in PSUM — W (113 MB/core) is read from HBM exactly once.
    The "@ L" branch is folded into the conv by precomputing ZcolL = Zcol @ L
    on-chip (Zcol never materialized; its tiles are slices of padded Z).
    After the convs: instance-norm + relu fused into one scalar-engine
    activation per tile; K and V are all-gathered; each core computes its
    row-block A_loc = exp(Q_loc K_full^T) and the transposed block
    At = exp(K_full Q_loc^T); then
        out = 0.5*rinv*(At^T-contract V_full) + 0.5*ReduceScatter(A_loc^T
              row-scaled V_loc)
    which realizes the symmetrized row-normalized attention exactly.
"""

import os
import sys

import numpy as np

sys.path.insert(0, "/opt/trn_rl_repo")

import orjson

import concourse.bass as bass
import concourse.mybir as mybir
from concourse import masks, tile
from concourse.bass_utils import run_bass_kernel_spmd

# ---------------------------------------------------------------- waitfix ---
# This neuronxcc build allows only ONE sync wait per instruction;
# TileContext emits instructions with several.  Rewrite the serialized BIR:
# hoist extra waits onto standalone NoOps inserted just before the
# instruction on the same engine (cumulative thresholds -> semantics kept).

_DMA_OPCODES = {
    "DMACopy", "DMATranspose", "TensorLoad", "TensorSave",
    "TriggeredCopy", "CollectiveCompute",
}
_wfix_counter = [0]


def _fix_block(instructions):
    out = []
    for ins in instructions:
        si = ins.get("sync_info")
        if not si:
            out.append(ins)
            continue
        waits = si.get("on_wait") or []
        updates = si.get("on_update") or []
        if len(waits) > 1:
            for w in waits[1:]:
                _wfix_counter[0] += 1
                out.append({
                    "engine": ins["engine"], "ins": [],
                    "name": f"WFIX-{_wfix_counter[0]}", "opcode": "NoOp",
                    "outs": [],
                    "sync_info": {"on_update": [], "on_wait": [w]},
                })
            si["on_wait"] = waits[:1]
        deferred = []
        if len(updates) > 1:
            assert ins.get("opcode", "") not in _DMA_OPCODES, (
                f"multi-update on DMA opcode: {ins['name']}"
            )
            si["on_update"] = updates[:1]
            for u in updates[1:]:
                _wfix_counter[0] += 1
                deferred.append({
                    "engine": ins["engine"], "ins": [],
                    "name": f"WFIX-{_wfix_counter[0]}", "opcode": "NoOp",
                    "outs": [],
                    "sync_info": {"on_update": [u], "on_wait": []},
                })
        out.append(ins)
        out.extend(deferred)
    return out


def _fix_bir_json_bytes(data: bytes) -> bytes:
    d = orjson.loads(data)
    for func in d.get("functions", []):
        for bb in func.get("blocks", []):
            bb["instructions"] = _fix_block(bb["instructions"])
    return orjson.dumps(d)


if not getattr(bass.Bass, "_waitfix_installed", False):
    _orig_to_json_bytes = bass.Bass.to_json_bytes

    def _patched_to_json_bytes(self) -> bytes:
        return _fix_bir_json_bytes(_orig_to_json_bytes(self))

    bass.Bass.to_json_bytes = _patched_to_json_bytes
    bass.Bass._waitfix_installed = True

# Synthesize the missing ``antenv.axon_hooks`` module so that
# ``run_bass_kernel_spmd(trace=True)`` can drive NTFF profiling through the
# axon PJRT plugin (the boot-time registration degrades silently when the
# module is absent).  Harmless when tracing is never requested.
try:
    import types

    import antenv

    if not hasattr(antenv, "axon_hooks"):
        _hooks_mod = types.ModuleType("antenv.axon_hooks")
        _ntff_hook = [None]
        _hooks_mod.set_axon_ntff_profile_hook = lambda h: _ntff_hook.__setitem__(0, h)
        _hooks_mod.get_axon_ntff_profile_hook = lambda: _ntff_hook[0]
        sys.modules["antenv.axon_hooks"] = _hooks_mod
        antenv.axon_hooks = _hooks_mod
        from trn_agent_boot.trn_boot import _ntff_profile_via_ctypes

        _hooks_mod.set_axon_ntff_profile_hook(
            _ntff_profile_via_ctypes("/opt/axon/libaxon_pjrt.so"))

    import concourse.bass_utils as _bu

    _bu.upload_artifacts = lambda tmpdir: tmpdir  # no fish share in container
except Exception:  # pragma: no cover - profiling is best-effort
    pass

# ------------------------------------------------------------- constants ---

NHID = 2048
NOPEN = 2048
N = 48          # spatial length
KD = 9          # conv kernel width
PAD = 4
NP = N + 2 * PAD            # 56 padded spatial
EPS = 1e-5
CORES = 8
OLOC = NHID // CORES        # 256 output channels per core
NGRP = 6                    # (b, r) conv groups
OCOLS = NGRP * OLOC         # 1536 W^T columns per core
KTOT = KD * NOPEN           # 18432 contraction length
NKT = KTOT // 128           # 144 k-tiles
ISUB = NOPEN // 128         # 16 i-subtiles
CKA = 4                     # k-tiles per W DMA chunk (sweep A, 1 MB bf16)
NCHA = NKT // CKA           # 36 chunks (sweep A)
CKB = 8                     # k-tiles per W DMA chunk (sweep B, 1 MB bf16)
NCHB = NKT // CKB           # 18 chunks (sweep B)
ACOLS = 4 * OLOC            # sweep A (Q,K): 1024 W^T cols per k-row
BCOLS = 2 * OLOC            # sweep B (V):    512 W^T cols per k-row
F32 = mybir.dt.float32
F32R = mybir.dt.float32r
BF16 = mybir.dt.bfloat16


DEBUG = bool(int(os.environ.get("KERNEL_DEBUG", "0")))


def _build_nc():
    nc = bass.Bass()

    wta_d = nc.declare_dram_parameter(
        "wta", [NCHA, 128, CKA * ACOLS], BF16, isOutput=False)
    wtb_d = nc.declare_dram_parameter(
        "wtb", [NCHB, 128, CKB * BCOLS], BF16, isOutput=False)
    z_d = nc.declare_dram_parameter("z", [NOPEN, N], F32, isOutput=False)
    l_d = nc.declare_dram_parameter("l", [N, N], F32, isOutput=False)
    out_d = nc.declare_dram_parameter("out", [OLOC, N], F32, isOutput=True)
    if DEBUG:
        dbg_conv = nc.declare_dram_parameter(
            "dbg_conv", [12, 128, N], F32, isOutput=True)
        dbg_qkv = nc.declare_dram_parameter(
            "dbg_qkv", [3, 2, 128, N], F32, isOutput=True)
        dbg_rs = nc.declare_dram_parameter(
            "dbg_rs", [2, 128, 1], F32, isOutput=True)
        dbg_zcl = nc.declare_dram_parameter(
            "dbg_zcl", [128, NKT * N], F32, isOutput=True)

    with tile.TileContext(nc) as tc:
        with (
            tc.tile_pool(name="pers", bufs=1) as pers,
            tc.tile_pool(name="wpool", bufs=6) as wpool,
            tc.tile_pool(name="wpoolb", bufs=5) as wpoolb,
            tc.tile_pool(name="stats", bufs=1) as stats,
            tc.tile_pool(name="pacc", bufs=2, space="PSUM") as pacc,
            tc.tile_pool(name="ptrans", bufs=2, space="PSUM") as ptrans,
            tc.tile_pool(name="dram", bufs=1, space="DRAM") as dram,
        ):
            # ---------------- prologue: Z, L, identity, ZpadT, ZcolL -------
            ident = pers.tile([128, 128], F32, tag="ident")
            masks.make_identity(nc, ident[:])

            # rendezvous: absorb cross-core launch/progress skew while the
            # W stream runs, so the mid-kernel all-gathers don't stall
            rg = [list(range(CORES))]
            rdv_in = dram.tile([2, 4], F32, tag="rdv_in")
            rdv_out = dram.tile([16, 4], F32, tag="rdv_out", addr_space="Shared")
            nc.gpsimd.collective_compute(
                "AllGather", mybir.AluOpType.bypass,
                replica_groups=rg, ins=[rdv_in.opt()], outs=[rdv_out.opt()])

            # padded Z: 16 tiles [128, 56] side by side
            zpad = pers.tile([128, ISUB * NP], F32, tag="zpad")
            nc.vector.memset(zpad[:], 0.0)
            zpad_v = zpad[:].rearrange("p (a c) -> p a c", c=NP)
            nc.sync.dma_start(
                out=zpad_v[:, :, PAD:PAD + N],
                in_=z_d[:].rearrange("(a p) n -> p a n", p=128),
            )

            # L padded into 9 shifted copies: lpad[t:t+48, t*48:(t+1)*48] = L
            lpad = pers.tile([128, KD * N], F32, tag="lpad")
            nc.vector.memset(lpad[:], 0.0)
            for t in range(KD):
                nc.sync.dma_start(out=lpad[t:t + N, t * N:(t + 1) * N], in_=l_d[:])

            # ZpadT [56, 2048] via PE transposes of the 16 padded tiles
            zpadT = pers.tile([128, NOPEN], F32, tag="zpadT")
            for s in range(ISUB):
                ps = ptrans.tile([128, 128], F32, tag="ptrans")
                nc.tensor.transpose(
                    ps[0:NP, :], zpad[:, s * NP:(s + 1) * NP], ident[:])
                zp_copy = (nc.scalar.copy if s % 2 == 0
                           else nc.vector.tensor_copy)
                zp_copy(zpadT[0:NP, s * 128:(s + 1) * 128], ps[0:NP, :])

            # F32R copy of zpad for the conv stationaries
            zpadr = pers.tile([128, ISUB * NP], BF16, tag="zpadr")
            nc.vector.tensor_copy(zpadr[:], zpad[:])

            # ZcolL[k, n'] = sum_n Zpad[i, n+t] L[n, n'] — batched per i-subtile
            # (all 9 shifts in one N=432 matmul); stored isub-major:
            # slice for (t, s) lives at (s*KD + t) * N
            zcolL = pers.tile([128, ISUB * KD * N], BF16, tag="zcolL")
            for s in range(ISUB):
                ps = ptrans.tile([128, KD * N], F32, tag="pzcl", name=f"pzcl{s}", bufs=2)
                nc.tensor.matmul(
                    ps[:, 0:KD * N],
                    zpadT[0:NP, s * 128:(s + 1) * 128],
                    lpad[0:NP, :],
                    start=True, stop=True,
                )
                # alternate engines to halve the copy-chain latency
                eng_copy = (nc.scalar.copy if s % 2 == 0
                            else nc.vector.tensor_copy)
                eng_copy(zcolL[:, s * KD * N:(s + 1) * KD * N], ps[:, 0:KD * N])

            if DEBUG:
                nc.sync.dma_start(out=dbg_zcl[:], in_=zcolL[:])

            # ---------------- conv: stream W as the MOVING operand ---------
            # lhsT (stationary) = [128, 48] Z window / ZcolL tile (40 ns
            # LDWEIGHTS); rhs = W^T columns streaming at 1 col/cycle.  Two
            # k-sweeps: A covers Q+K groups (cols [g0,g2 | g1,g3]), B covers V
            # (cols [g4 | g5]) so the K all-gather + attention prework hide
            # behind sweep B.  One PSUM bank per group accumulator
            # (start=True clears has_written for the whole bank).
            relu_sc = pers.tile([128, 12 * N], F32, tag="relu_sc")
            yt_sb = pers.tile([128, 6 * OLOC], F32, tag="yt_sb")
            qkv = [pers.tile([128, 2 * N], F32, tag=f"qkv{b}", name=f"qkv{b}")
                   for b in range(3)]

            def sweep_epilogue(entries, label):
                """entries: list of (g, acc_ap[48, 256]).  Transpose each
                half to [128, 48], then batched instance-norm stats (one
                vector op per stage across all slots) + fused relu."""
                nslot = 2 * len(entries)
                xc = stats.tile([128, nslot * N], F32, tag=f"xc{label}",
                                name=f"xc{label}")
                slots = []
                for idx, (g, acc_ap) in enumerate(entries):
                    nc.scalar.copy(
                        yt_sb[0:N, g * OLOC:(g + 1) * OLOC], acc_ap)
                    for h in range(2):
                        ot = g * 2 + h
                        slot = idx * 2 + h
                        ps2 = ptrans.tile([128, 128], F32, tag="ptrans",
                                          name=f"tp{ot}")
                        nc.tensor.transpose(
                            ps2[:, 0:N],
                            yt_sb[0:N, g * OLOC + h * 128:
                                  g * OLOC + (h + 1) * 128],
                            ident[0:N, 0:N])
                        nc.scalar.copy(xc[:, slot * N:(slot + 1) * N],
                                       ps2[:, 0:N])
                        if DEBUG:
                            nc.scalar.dma_start(
                                out=dbg_conv[ot],
                                in_=xc[:, slot * N:(slot + 1) * N])
                        slots.append((ot, slot))
                sm = stats.tile([128, nslot], F32, tag=f"sm{label}",
                                name=f"sm{label}")
                sq = stats.tile([128, nslot], F32, tag=f"sq{label}",
                                name=f"sq{label}")
                scr = stats.tile([128, nslot * N], F32, tag=f"scr{label}",
                                 name=f"scr{label}")
                for ot, slot in slots:
                    nc.vector.reduce_sum(
                        sm[:, slot:slot + 1], xc[:, slot * N:(slot + 1) * N],
                        axis=mybir.AxisListType.X)
                nc.vector.tensor_tensor(scr[:], xc[:], xc[:],
                                        op=mybir.AluOpType.mult)
                for ot, slot in slots:
                    nc.vector.reduce_sum(
                        sq[:, slot:slot + 1], scr[:, slot * N:(slot + 1) * N],
                        axis=mybir.AxisListType.X)
                mean = stats.tile([128, nslot], F32, tag=f"mean{label}",
                                  name=f"mean{label}")
                var = stats.tile([128, nslot], F32, tag=f"var{label}",
                                 name=f"var{label}")
                std = stats.tile([128, nslot], F32, tag=f"std{label}",
                                 name=f"std{label}")
                rsv = stats.tile([128, nslot], F32, tag=f"rsv{label}",
                                 name=f"rsv{label}")
                nb = stats.tile([128, nslot], F32, tag=f"nb{label}",
                                name=f"nb{label}")
                nc.vector.tensor_scalar_mul(mean[:], sm[:], 1.0 / N)
                nc.vector.tensor_scalar_mul(sq[:], sq[:], 1.0 / N)
                nc.vector.tensor_tensor(var[:], mean[:], mean[:],
                                        op=mybir.AluOpType.mult)
                nc.vector.tensor_tensor(var[:], sq[:], var[:],
                                        op=mybir.AluOpType.subtract)
                nc.vector.tensor_scalar_add(var[:], var[:], EPS)
                nc.scalar.sqrt(std[:], var[:])
                nc.vector.reciprocal(rsv[:], std[:])
                nc.vector.tensor_tensor(nb[:], mean[:], rsv[:],
                                        op=mybir.AluOpType.mult)
                nc.vector.tensor_scalar_mul(nb[:], nb[:], -1.0)
                for ot, slot in slots:
                    nc.scalar.activation(
                        relu_sc[:, ot * N:(ot + 1) * N],
                        xc[:, slot * N:(slot + 1) * N],
                        mybir.ActivationFunctionType.Relu,
                        bias=nb[:, slot:slot + 1], scale=rsv[:, slot:slot + 1])

            def qkv_add(b):
                for h in range(2):
                    ot0 = (2 * b) * 2 + h        # r = 0
                    ot1 = (2 * b + 1) * 2 + h    # r = 1
                    nc.vector.tensor_tensor(
                        qkv[b][:, h * N:(h + 1) * N],
                        relu_sc[:, ot0 * N:(ot0 + 1) * N],
                        relu_sc[:, ot1 * N:(ot1 + 1) * N],
                        op=mybir.AluOpType.add)
                    if DEBUG:
                        nc.scalar.dma_start(
                            out=dbg_qkv[b, h],
                            in_=qkv[b][:, h * N:(h + 1) * N])

            # ---- sweep A: Q + K (cols [g0,g2 | g1,g3], one N=512 matmul
            # per stationary per k-tile; acc r0/r1 each fill one PSUM bank)
            accA = [pacc.tile([128, 2 * OLOC], F32, tag="accw", name=f"accA{i}")
                    for i in range(2)]
            for gch in range(NCHA):
                wt = wpool.tile([128, CKA * ACOLS], BF16, tag="wt",
                                name=f"wta{gch}")
                nc.sync.dma_start(out=wt[:], in_=wta_d[gch])
                for j in range(CKA):
                    kt = gch * CKA + j
                    t, s = kt // ISUB, kt % ISUB
                    lhs0 = zpadr[:, s * NP + t: s * NP + t + N]
                    lhs1 = zcolL[:, (s * KD + t) * N:(s * KD + t + 1) * N]
                    base = j * ACOLS
                    nc.tensor.matmul(
                        accA[0][0:N, :], lhs0, wt[:, base: base + 512],
                        start=(kt == 0), stop=(kt == NKT - 1))
                    nc.tensor.matmul(
                        accA[1][0:N, :], lhs1, wt[:, base + 512: base + 1024],
                        start=(kt == 0), stop=(kt == NKT - 1))

            # epilogue for the whole A sweep (K + V); both all-gathers
            # issue now and hide behind sweep B (Q)
            sweep_epilogue([(2, accA[0][0:N, 0:OLOC]),
                            (4, accA[0][0:N, OLOC:2 * OLOC]),
                            (3, accA[1][0:N, 0:OLOC]),
                            (5, accA[1][0:N, OLOC:2 * OLOC])], "A")
            qkv_add(1)
            qkv_add(2)
            qloc, kloc, vloc = qkv

            kb = dram.tile([OLOC, N], F32, tag="kb")
            vb = dram.tile([OLOC, N], F32, tag="vb")
            kg = dram.tile([NHID, N], F32, tag="kg", addr_space="Shared")
            vg = dram.tile([NHID, N], F32, tag="vg", addr_space="Shared")
            nc.scalar.dma_start(
                out=kb[:].rearrange("(a p) n -> p a n", p=128),
                in_=kloc[:].rearrange("p (a n) -> p a n", n=N))
            nc.gpsimd.collective_compute(
                "AllGather", mybir.AluOpType.bypass,
                replica_groups=rg, ins=[kb.opt()], outs=[kg.opt()])
            nc.scalar.dma_start(
                out=vb[:].rearrange("(a p) n -> p a n", p=128),
                in_=vloc[:].rearrange("p (a n) -> p a n", n=N))
            nc.gpsimd.collective_compute(
                "AllGather", mybir.AluOpType.bypass,
                replica_groups=rg, ins=[vb.opt()], outs=[vg.opt()])

            # ---- sweep B: Q ----
            accB = [pacc.tile([128, OLOC], F32, tag="acc", name=f"accB{i}")
                    for i in range(2)]  # order: g0, g1

            def sweep_b(c0, c1):
                for gch in range(c0, c1):
                    wt = wpoolb.tile([128, CKB * BCOLS], BF16, tag="wtb",
                                     name=f"wtb{gch}")
                    nc.sync.dma_start(out=wt[:], in_=wtb_d[gch])
                    for j in range(CKB):
                        kt = gch * CKB + j
                        t, s = kt // ISUB, kt % ISUB
                        lhs0 = zpadr[:, s * NP + t: s * NP + t + N]
                        lhs1 = zcolL[:, (s * KD + t) * N:(s * KD + t + 1) * N]
                        base = j * BCOLS
                        nc.tensor.matmul(
                            accB[0][0:N, :], lhs0,
                            wt[:, base: base + OLOC],
                            start=(kt == 0), stop=(kt == NKT - 1))
                        nc.tensor.matmul(
                            accB[1][0:N, :], lhs1,
                            wt[:, base + OLOC: base + 2 * OLOC],
                            start=(kt == 0), stop=(kt == NKT - 1))

            sweep_b(0, 9)

            # kT transposes + K/V landing overlap sweep B on the PE
            kfull = pers.tile([128, 16 * N], F32, tag="kfull")
            nc.gpsimd.dma_start(
                out=kfull[:].rearrange("p (a n) -> p a n", n=N),
                in_=kg[:].rearrange("(a p) n -> p a n", p=128))
            kT = pers.tile([128, NHID], F32R, tag="kT")
            for jt in range(16):
                ps = ptrans.tile([128, 128], F32, tag="ptrans")
                nc.tensor.transpose(
                    ps[0:N, :], kfull[:, jt * N:(jt + 1) * N], ident[:])
                nc.scalar.copy(kT[0:N, jt * 128:(jt + 1) * 128], ps[0:N, :])
            vfull = pers.tile([128, 16 * N], F32, tag="vfull")
            nc.gpsimd.dma_start(
                out=vfull[:].rearrange("p (a n) -> p a n", n=N),
                in_=vg[:].rearrange("(a p) n -> p a n", p=128))

            sweep_b(9, NCHB)

            sweep_epilogue([(0, accB[0][0:N, :]),
                            (1, accB[1][0:N, :])], "B")
            qkv_add(0)

            qT = pers.tile([128, 2 * 128], F32R, tag="qT")
            for h in range(2):
                ps = ptrans.tile([128, 128], F32, tag="ptrans")
                nc.tensor.transpose(
                    ps[0:N, :], qloc[:, h * N:(h + 1) * N], ident[:])
                nc.scalar.copy(qT[0:N, h * 128:(h + 1) * 128], ps[0:N, :])

            # A = exp(Q K^T) rows + rowsum
            a_sb = [pers.tile([128, NHID], F32, tag=f"a{m}", name=f"a{m}")
                    for m in range(2)]
            rinvh = []
            for m in range(2):
                rspart = stats.tile([128, 4], F32, tag=f"rsp{m}", name=f"rsp{m}")
                for jc in range(4):
                    ps = ptrans.tile([128, 512], F32, tag="ptrans")
                    nc.tensor.matmul(
                        ps[:, 0:512],
                        qT[0:N, m * 128:(m + 1) * 128],
                        kT[0:N, jc * 512:(jc + 1) * 512],
                        start=True, stop=True)
                    nc.scalar.activation(
                        a_sb[m][:, jc * 512:(jc + 1) * 512], ps[:, 0:512],
                        mybir.ActivationFunctionType.Exp,
                        accum_out=rspart[:, jc:jc + 1])
                rowsum = stats.tile([128, 1], F32, tag=f"rowsum{m}", name=f"rowsum{m}")
                nc.vector.reduce_sum(rowsum[:], rspart[:], axis=mybir.AxisListType.X)
                rinv = stats.tile([128, 1], F32, tag=f"rinv{m}", name=f"rinv{m}")
                nc.vector.reciprocal(rinv[:], rowsum[:])
                rh = stats.tile([128, 1], F32, tag=f"rinvh{m}", name=f"rinvh{m}")
                nc.vector.tensor_scalar_mul(rh[:], rinv[:], 0.5)
                rinvh.append((rinv, rh))
                if DEBUG:
                    nc.scalar.dma_start(out=dbg_rs[m], in_=rowsum[:])

            # At = exp(K Q^T)  [2048, 256]
            at_sb = pers.tile([128, 16 * 256], F32, tag="at")
            for jt in range(16):
                ps = ptrans.tile([128, 256], F32, tag="ptrans")
                nc.tensor.matmul(
                    ps[:, 0:256],
                    kT[0:N, jt * 128:(jt + 1) * 128],
                    qT[0:N, 0:256],
                    start=True, stop=True)
                nc.scalar.activation(
                    at_sb[:, jt * 256:(jt + 1) * 256], ps[:, 0:256],
                    mybir.ActivationFunctionType.Exp)

            # ---------------- U = A_loc^T (rinv*V_loc); ReduceScatter ------
            vr = pers.tile([128, 2 * N], F32, tag="vr")
            for m in range(2):
                nc.vector.tensor_scalar_mul(
                    vr[:, m * N:(m + 1) * N], vloc[:, m * N:(m + 1) * N],
                    rinvh[m][0][:])
            u_sb = pers.tile([128, 16 * N], F32, tag="u")
            for jt in range(16):
                ps = ptrans.tile([128, 128], F32, tag="ptrans")
                for m in range(2):
                    nc.tensor.matmul(
                        ps[:, 0:N],
                        a_sb[m][:, jt * 128:(jt + 1) * 128],
                        vr[:, m * N:(m + 1) * N],
                        start=(m == 0), stop=(m == 1))
                nc.vector.tensor_copy(u_sb[:, jt * N:(jt + 1) * N], ps[:, 0:N])

            ub = dram.tile([NHID, N], F32, tag="ub")
            rsb = dram.tile([OLOC, N], F32, tag="rsb")
            nc.scalar.dma_start(
                out=ub[:].rearrange("(a p) n -> p a n", p=128),
                in_=u_sb[:].rearrange("p (a n) -> p a n", n=N))
            nc.gpsimd.collective_compute(
                "ReduceScatter", mybir.AluOpType.add,
                replica_groups=rg, ins=[ub.opt()], outs=[rsb.opt()])
            rs_sb = pers.tile([128, 2 * N], F32, tag="rs_sb")
            nc.gpsimd.dma_start(
                out=rs_sb[:].rearrange("p (a n) -> p a n", n=N),
                in_=rsb[:].rearrange("(a p) n -> p a n", p=128))

            # ---------------- out1 = rinv * (At^T-contract V_full) ---------
            fin = pers.tile([128, 2 * N], F32, tag="fin")
            rs_half = pers.tile([128, 2 * N], F32, tag="rs_half")
            for m in range(2):
                ps = ptrans.tile([128, 128], F32, tag="ptrans")
                for jt in range(16):
                    nc.tensor.matmul(
                        ps[:, 0:N],
                        at_sb[:, jt * 256 + m * 128: jt * 256 + (m + 1) * 128],
                        vfull[:, jt * N:(jt + 1) * N],
                        start=(jt == 0), stop=(jt == 15))
                nc.vector.tensor_scalar_mul(
                    rs_half[:, m * N:(m + 1) * N], rs_sb[:, m * N:(m + 1) * N], 0.5)
                nc.vector.scalar_tensor_tensor(
                    out=fin[:, m * N:(m + 1) * N],
                    in0=ps[:, 0:N],
                    scalar=rinvh[m][1][:],
                    in1=rs_half[:, m * N:(m + 1) * N],
                    op0=mybir.AluOpType.mult,
                    op1=mybir.AluOpType.add)

            nc.scalar.dma_start(
                out=out_d[:].rearrange("(a p) n -> p a n", p=128),
                in_=fin[:].rearrange("p (a n) -> p a n", n=N))

    return nc


_NC_CACHE = None


def _get_nc():
    global _NC_CACHE
    if _NC_CACHE is None:
        _NC_CACHE = _build_nc()
    return _NC_CACHE


def _prep_w(W: np.ndarray) -> list[np.ndarray]:
    """Per-core streaming slab: [NCHUNK, 128, CHUNK_KT*OCOLS] f32.

    col = (b*2+r)*256 + o_loc ; row k = t*2048 + i ; chunk-major partitions.
    """
    import ml_dtypes

    def sweep(wt, groups, cols, nch, ck):
        blk = np.concatenate(
            [wt[:, g * OLOC:(g + 1) * OLOC] for g in groups], axis=1)
        blk = blk.reshape(nch, ck, 128, cols).transpose(0, 2, 1, 3)
        return np.ascontiguousarray(
            blk.reshape(nch, 128, ck * cols).astype(ml_dtypes.bfloat16))

    shards = []
    for c in range(CORES):
        wc = W[:, :, c * OLOC:(c + 1) * OLOC, :, :]        # [3,2,256,2048,9]
        wt = wc.transpose(4, 3, 0, 1, 2).reshape(KTOT, OCOLS)
        shards.append((sweep(wt, [2, 4, 3, 5], ACOLS, NCHA, CKA),
                       sweep(wt, [0, 1], BCOLS, NCHB, CKB)))
    return shards


def kernel(Z: np.ndarray, L: np.ndarray, W: np.ndarray) -> np.ndarray:
    nc = _get_nc()
    wts = _prep_w(np.asarray(W, dtype=np.float32))
    z = np.ascontiguousarray(Z, dtype=np.float32)
    l = np.ascontiguousarray(L, dtype=np.float32)
    in_maps = [{"wta": wts[c][0], "wtb": wts[c][1], "z": z, "l": l}
               for c in range(CORES)]
    trace = bool(int(os.environ.get("KERNEL_TRACE", "0")))
    kw = {}
    if trace and int(os.environ.get("KERNEL_TRACE_ALL", "0")):
        kw["trace_cores"] = list(range(CORES))
    res = run_bass_kernel_spmd(nc, in_maps, list(range(CORES)), trace=trace, **kw)
    kernel.last_result = res
    out = np.concatenate([res.results[c]["out"] for c in range(CORES)], axis=0)
    return out
